# revision 14
# baseline (speedup 1.0000x reference)
"""DrugGraphEmbedding (2x SAGEConv + sym-Laplacian features + mean-pool) on 8 trn2 cores.

Strategy: node-shard the 1024 graphs (128 graphs = 6144 nodes per core).
Aggregations (Laplacian smoothing, SAGE mean over incoming edges) run as
dma_gather of source rows + one-hot PE matmuls that scatter 128-edge chunks
into 128-destination PSUM windows.

v2 restructure vs baseline:
  - Cross-core exchange uses CHUNKED AllGathers (4 chunks, quarter-major
    layout) fired progressively as lap / hidden blocks complete.
  - Dense SAGE root-terms (x@Wr) and the following layer's l-terms are
    software-pipelined INTO the gather/aggregate loops so the PE fills
    AllGather time and gpsimd gather time.
  - Gather idx streams use two OVERLAPPING int16 tables [0,32K) and
    [16K,48K) with per-(core,window) balanced split -> fewer padded chunks.
  - Gather calls batch 8 chunks (1024 idxs) to amortize SWDGE fixed cost.
  - Constants are host-packed (contiguous loads); idx tables load first.
Everything computes in fp16 with f32 PSUM accumulation.
"""

import numpy as np

B, S, D = 1024, 48, 256
GDIM = 512
N = B * S            # 49152
E = 4 * N            # 196608
NCORES = 8
NLOC = N // NCORES   # 6144
WIN = 128            # dst nodes per PSUM window
NWIN = NLOC // WIN   # 48
NBLK = NWIN // 4     # 12 (512-node dense blocks)
QCHUNK = NLOC // 4   # 1536 rows per AllGather chunk (per core)
T0_END = 32768       # gather table 0 = rows [0, 32768)
T1_OFF = 16384       # gather table 1 = rows [16384, 49152)


def _cdiv(a, b):
    return -(-a // b)


def _pack_idx(idx_stream):
    """int16 stream -> [128, len/16] wrapped tile (16 partitions, replicated x8)."""
    L = len(idx_stream)
    assert L % 16 == 0
    w = idx_stream.reshape(L // 16, 16).T  # [16, L/16]
    return np.tile(w, (8, 1)).astype(np.int16)


def _build_streams(dst, src, wgt):
    """Pad edges into per-(core, window, table) chunk streams with a shared
    chunks-per-window structure (SPMD: same program on every core).

    Tables OVERLAP: table0 = rows [0, T0_END), table1 = rows [T1_OFF, N).
    Edges with src in the overlap region [T1_OFF, T0_END) are assigned
    per-(core, window) to balance chunk counts (minimizes padding)."""
    gwin = dst // WIN                    # [E'] global window id 0..NCORES*NWIN-1
    cat = np.where(src < T1_OFF, 0, np.where(src < T0_END, 1, 2))  # lo/mid/hi
    order = np.lexsort((src, cat, gwin))
    dst_s, src_s = dst[order], src[order]
    wgt_s = wgt[order] if wgt is not None else None

    NW = NCORES * NWIN
    cnt = np.bincount(gwin, minlength=NW).reshape(NCORES, NWIN)
    cntlo = np.bincount(gwin[cat == 0], minlength=NW).reshape(NCORES, NWIN)
    cnthi = np.bincount(gwin[cat == 2], minlength=NW).reshape(NCORES, NWIN)

    K = np.maximum(_cdiv(cnt, 128).max(axis=0), 2)        # [NWIN] chunks/window
    k0 = np.maximum(_cdiv(cntlo, 128).max(axis=0), 1)     # table0 chunks
    k1 = np.maximum(K - k0, np.maximum(_cdiv(cnthi, 128).max(axis=0), 1))

    nch = [int(k0.sum()), int(k1.sum())]
    base = np.zeros((NWIN, 2), np.int64)
    base[1:, 0] = np.cumsum(k0[:-1])
    base[1:, 1] = np.cumsum(k1[:-1])
    cpw = np.stack([k0, k1], axis=1)

    flat_cnt = cnt.reshape(-1)
    starts = np.concatenate([[0], np.cumsum(flat_cnt)[:-1]]).reshape(NCORES, NWIN)
    cnt_nonhi = cnt - cnthi

    per_core = []
    for c in range(NCORES):
        idx_h = [np.zeros(nch[h] * 128, np.int64) for h in (0, 1)]
        dl_h = [np.full(nch[h] * 128, -1.0, np.float16) for h in (0, 1)]
        wg_h = [np.zeros(nch[h] * 128, np.float16) for h in (0, 1)]
        for w in range(NWIN):
            s0 = int(starts[c, w])
            n = int(cnt[c, w])
            a = min(128 * int(k0[w]), int(cnt_nonhi[c, w]))
            sw_src = src_s[s0 : s0 + n]
            sw_dst = dst_s[s0 : s0 + n]
            sw_w = wgt_s[s0 : s0 + n] if wgt_s is not None else None
            for h, (i0, i1) in enumerate(((0, a), (a, n))):
                m = i1 - i0
                assert m <= 128 * int(cpw[w, h]), (c, w, h, m, cpw[w, h])
                p0 = int(base[w, h]) * 128
                ss = sw_src[i0:i1] - (T1_OFF if h else 0)
                if m:
                    assert ss.min() >= 0 and ss.max() < 32768
                idx_h[h][p0 : p0 + m] = ss
                dl_h[h][p0 : p0 + m] = (sw_dst[i0:i1] % WIN).astype(np.float16)
                if sw_w is not None:
                    wg_h[h][p0 : p0 + m] = sw_w[i0:i1].astype(np.float16)
                else:
                    wg_h[h][p0 : p0 + m] = 1.0
        per_core.append({
            "idx": [_pack_idx(idx_h[h].astype(np.int16)) for h in (0, 1)],
            "dstl": [np.ascontiguousarray(dl_h[h].reshape(nch[h], 128).T) for h in (0, 1)],
            "wgt": [np.ascontiguousarray(wg_h[h].reshape(nch[h], 128).T) for h in (0, 1)],
        })

    return {"cpw": cpw, "base": base, "nch": nch}, per_core


def _host_prep(edge_index):
    row = np.asarray(edge_index[0], np.int64)
    col = np.asarray(edge_index[1], np.int64)

    deg = np.bincount(row, minlength=N).astype(np.float64)
    dinv = (deg > 0) / np.sqrt(np.maximum(deg, 1.0))
    cnt = np.bincount(col, minlength=N).astype(np.float64)
    cinv = 1.0 / np.maximum(cnt, 1.0)

    lap_w = -(dinv[row] * dinv[col])
    lap_struct, lap_pc = _build_streams(row, col, lap_w)     # dst=row, src=col

    # conv gathers read the AllGathered tables, which land in quarter-major
    # layout: row r -> 12288*q + 1536*c + pos  (c = owner core, q = quarter)
    r = np.arange(N, dtype=np.int64)
    c_of, rem = np.divmod(r, NLOC)
    q_of, pos = np.divmod(rem, QCHUNK)
    qperm = q_of * (NCORES * QCHUNK) + c_of * QCHUNK + pos
    conv_struct, conv_pc = _build_streams(col, qperm[row], None)  # dst=col, src=row

    cinv_tiles = []
    for c in range(NCORES):
        ci = cinv[c * NLOC : (c + 1) * NLOC].reshape(NWIN, WIN).T  # [128, NWIN]
        cinv_tiles.append(np.ascontiguousarray(ci).astype(np.float32))

    pool_dstl = np.zeros((128, NWIN), np.float16)
    for nt in range(NWIN):
        pool_dstl[:, nt] = ((nt * 128 + np.arange(128)) // S).astype(np.float16)

    iota = np.tile(np.arange(128, dtype=np.float16)[None, :], (128, 1))
    return {
        "lap": (lap_struct, lap_pc),
        "conv": (conv_struct, conv_pc),
        "cinv": cinv_tiles,
        "pool_dstl": pool_dstl,
        "iota": iota,
    }


def _build_program(lap_struct, conv_struct):
    import os

    import concourse.bass as bass
    import concourse.bacc as bacc
    import concourse.mybir as mybir
    from concourse.tile import TileContext

    phases = int(os.environ.get("KPHASES", "5"))
    GRP = int(os.environ.get("KGRP", "8"))
    single_packet = os.environ.get("KSP", "0") == "1"

    fp16 = mybir.dt.float16
    f32 = mybir.dt.float32
    i16 = mybir.dt.int16

    nc = bacc.Bacc(
        "TRN2",
        target_bir_lowering=False,
        debug=False,
        num_devices=NCORES,
        dynamic_dma_scratch_size=24576,
        num_swdge_queues=4,
    )

    # ---- inputs -----------------------------------------------------------
    x16 = nc.dram_tensor("x16", [N, D], fp16, kind="ExternalInput")
    x16own = nc.dram_tensor("x16own", [NLOC, D], fp16, kind="ExternalInput")
    xT16 = nc.dram_tensor("xT16", [D, NLOC], fp16, kind="ExternalInput")
    xw_in = nc.dram_tensor("xwrap", [128, NWIN * D], fp16, kind="ExternalInput")
    iota_in = nc.dram_tensor("iota", [128, 128], fp16, kind="ExternalInput")
    ones_in = nc.dram_tensor("ones1", [1, 128], fp16, kind="ExternalInput")
    cinv_in = nc.dram_tensor("cinv", [128, NWIN], f32, kind="ExternalInput")
    pdstl_in = nc.dram_tensor("pool_dstl", [128, NWIN], fp16, kind="ExternalInput")

    wts_in = {}
    for nm in ("Wr1T", "Wl1T", "Wr2T", "Wl2T"):
        wts_in[nm] = nc.dram_tensor(nm, [128, 4 * GDIM], fp16, kind="ExternalInput")
    b1_in = nc.dram_tensor("b1T", [1, GDIM], fp16, kind="ExternalInput")
    b2_in = nc.dram_tensor("b2T", [1, GDIM], fp16, kind="ExternalInput")

    lap_nch, conv_nch = lap_struct["nch"], conv_struct["nch"]
    lap_idx_in, conv_idx_in, lap_dstl_in, lap_w_in, conv_dstl_in = [], [], [], [], []
    for h in (0, 1):
        lap_idx_in.append(
            nc.dram_tensor(f"lap_idx{h}", [128, lap_nch[h] * 8], i16, kind="ExternalInput")
        )
        conv_idx_in.append(
            nc.dram_tensor(f"conv_idx{h}", [128, conv_nch[h] * 8], i16, kind="ExternalInput")
        )
        lap_dstl_in.append(
            nc.dram_tensor(f"lap_dstl{h}", [128, lap_nch[h]], fp16, kind="ExternalInput")
        )
        lap_w_in.append(
            nc.dram_tensor(f"lap_w{h}", [128, lap_nch[h]], fp16, kind="ExternalInput")
        )
        conv_dstl_in.append(
            nc.dram_tensor(f"conv_dstl{h}", [128, conv_nch[h]], fp16, kind="ExternalInput")
        )

    o_pool = nc.dram_tensor("o_pool", [128, GDIM], f32, kind="ExternalOutput")
    kdump = os.environ.get("KDUMP") == "1"
    dumps = {}
    if kdump:
        dumps["o_xc"] = nc.dram_tensor("o_xc", [NLOC, 2 * D], fp16, kind="ExternalOutput")
        dumps["o_m1"] = nc.dram_tensor("o_m1", [NLOC, GDIM], fp16, kind="ExternalOutput")
        dumps["o_h"] = nc.dram_tensor("o_h", [NLOC, GDIM], fp16, kind="ExternalOutput")
        dumps["o_m2"] = nc.dram_tensor("o_m2", [NLOC, GDIM], fp16, kind="ExternalOutput")
        dumps["o_r"] = nc.dram_tensor("o_r", [128, NWIN * GDIM], fp16, kind="ExternalOutput")

    # ---- internal DRAM ----------------------------------------------------
    # AllGather sources are split per quarter-chunk: Tile tracks DRAM deps at
    # whole-tensor granularity, so a single tensor would give later stores a
    # false WAR dependency on the in-flight chunk collective reading it.
    xcq = [nc.dram_tensor(f"xcq{q}", [QCHUNK, 2 * D], fp16) for q in range(4)]
    xcomb_full = nc.dram_tensor("xcomb_full", [N, 2 * D], fp16, addr_space="Shared")
    hq = [nc.dram_tensor(f"hq{q}", [QCHUNK, GDIM], fp16) for q in range(4)]
    h_full = nc.dram_tensor("h_full", [N, GDIM], fp16, addr_space="Shared")
    m1_dram = nc.dram_tensor("m1_dram", [NLOC, GDIM], fp16)
    m2_dram = nc.dram_tensor("m2_dram", [NLOC, GDIM], fp16)

    RG = [list(range(NCORES))]

    with TileContext(nc) as tc:
        with (
            tc.tile_pool(name="const", bufs=1) as cpool,
            tc.tile_pool(name="mlap", bufs=3) as mlap,
            tc.tile_pool(name="mconv", bufs=3) as mconv,
            tc.tile_pool(name="asg", bufs=4) as apool,
            tc.tile_pool(name="tT", bufs=8) as tpool,
            tc.tile_pool(name="o16", bufs=4) as opool,
            tc.tile_pool(name="gl", bufs=2) as gpool,
            tc.tile_pool(name="of32", bufs=1) as f32pool,
            tc.tile_pool(name="pagg", bufs=2, space="PSUM") as pagg,
            tc.tile_pool(name="pbig", bufs=2, space="PSUM") as pbig,
            tc.tile_pool(name="ppool", bufs=1, space="PSUM") as ppool,
        ):
            # ---- critical-path constants on sync queue -------------------
            iota = cpool.tile([128, 128], fp16, tag="iota")
            nc.sync.dma_start(out=iota[:], in_=iota_in[:])
            lap_idx_t, conv_idx_t, lap_dstl_t, lap_w_t, conv_dstl_t = [], [], [], [], []
            for h in (0, 1):
                t = cpool.tile([128, lap_nch[h] * 8], i16, tag=f"lidx{h}")
                nc.sync.dma_start(out=t[:], in_=lap_idx_in[h][:])
                lap_idx_t.append(t)
                t = cpool.tile([128, lap_nch[h]], fp16, tag=f"ldstl{h}")
                nc.sync.dma_start(out=t[:], in_=lap_dstl_in[h][:])
                lap_dstl_t.append(t)
                t = cpool.tile([128, lap_nch[h]], fp16, tag=f"lw{h}")
                nc.sync.dma_start(out=t[:], in_=lap_w_in[h][:])
                lap_w_t.append(t)
            for h in (0, 1):
                t = cpool.tile([128, conv_nch[h] * 8], i16, tag=f"cidx{h}")
                nc.sync.dma_start(out=t[:], in_=conv_idx_in[h][:])
                conv_idx_t.append(t)
                t = cpool.tile([128, conv_nch[h]], fp16, tag=f"cdstl{h}")
                nc.sync.dma_start(out=t[:], in_=conv_dstl_in[h][:])
                conv_dstl_t.append(t)

            # ---- bulk constants on scalar queue --------------------------
            ones1 = cpool.tile([1, 128], fp16, tag="ones1")
            nc.scalar.dma_start(out=ones1[:], in_=ones_in[:])
            cinv_t = cpool.tile([128, NWIN], f32, tag="cinv")
            nc.scalar.dma_start(out=cinv_t[:], in_=cinv_in[:])
            pdstl = cpool.tile([128, NWIN], fp16, tag="pdstl")
            nc.scalar.dma_start(out=pdstl[:], in_=pdstl_in[:])
            b1t = cpool.tile([1, GDIM], fp16, tag="b1")
            nc.scalar.dma_start(out=b1t[:], in_=b1_in[:])
            b2t = cpool.tile([1, GDIM], fp16, tag="b2")
            nc.scalar.dma_start(out=b2t[:], in_=b2_in[:])
            xw_all = cpool.tile([128, NWIN * D], fp16, tag="xw")
            nc.scalar.dma_start(out=xw_all[:], in_=xw_in[:])
            wt = {}
            for nm in ("Wr1T", "Wl1T", "Wr2T", "Wl2T"):
                t = cpool.tile([128, 4 * GDIM], fp16, tag=nm)
                nc.scalar.dma_start(out=t[:], in_=wts_in[nm][:])
                wt[nm] = t
            # x-half of xcomb (DRAM->DRAM, overlapped with lap gathers)
            for q in range(4):
                nc.scalar.dma_start(
                    out=xcq[q].ap()[:, 0:D],
                    in_=x16own.ap()[q * QCHUNK : (q + 1) * QCHUNK, :],
                )

            # pool one-hot: [128, NWIN, 128]
            pool_asg = cpool.tile([128, NWIN, 128], fp16, tag="pasg")
            nc.vector.tensor_tensor(
                out=pool_asg[:],
                in0=pdstl[:].to_broadcast([128, NWIN, 128]),
                in1=iota[:, None, :].to_broadcast([128, NWIN, 128]),
                op=mybir.AluOpType.is_equal,
            )

            # persistent root-term buffer (r1 then overwritten in place by r2)
            r_sbuf = cpool.tile([128, NWIN, GDIM], fp16, tag="rterm")

            qctr = [0]

            class AggPlan:
                """Just-in-time gather + assign-build for one chunk-stream pass."""

                def __init__(self, struct, idx_tiles, tabs, elem, grp, pool,
                             dstl_tiles, w_tiles):
                    self.struct = struct
                    self.idx_tiles = idx_tiles
                    self.tabs = tabs            # (table0_ap, table1_ap)
                    self.elem = elem
                    self.grp = grp
                    self.pool = pool
                    self.dstl_tiles = dstl_tiles
                    self.w_tiles = w_tiles
                    self.msgs = {}
                    self.asg = {}

                def _ensure(self, h, g):
                    if (h, g) in self.msgs:
                        return
                    c0 = g * self.grp
                    cn = min(self.grp, self.struct["nch"][h] - c0)
                    ni = cn * 128
                    tile = self.pool.tile([128, self.grp, self.elem], fp16, tag="msgs")
                    nc.gpsimd.dma_gather(
                        out_ap=tile[:, 0:cn, :],
                        in_ap=self.tabs[h],
                        idxs_ap=self.idx_tiles[h][:, c0 * 8 : (c0 + cn) * 8],
                        num_idxs=ni,
                        num_idxs_reg=ni,
                        elem_size=self.elem,
                        single_packet=single_packet,
                        queue_num=qctr[0] % 4,
                    )
                    qctr[0] += 1
                    self.msgs[(h, g)] = tile
                    t = apool.tile([128, self.grp, 128], fp16, tag="asg")
                    nc.vector.tensor_tensor(
                        out=t[:, 0:cn, :],
                        in0=self.dstl_tiles[h][:, c0 : c0 + cn].to_broadcast(
                            [128, cn, 128]
                        ),
                        in1=iota[:, None, :].to_broadcast([128, cn, 128]),
                        op=mybir.AluOpType.is_equal,
                    )
                    if self.w_tiles is not None:
                        nc.vector.tensor_tensor(
                            out=t[:, 0:cn, :],
                            in0=t[:, 0:cn, :],
                            in1=self.w_tiles[h][:, c0 : c0 + cn].to_broadcast(
                                [128, cn, 128]
                            ),
                            op=mybir.AluOpType.mult,
                        )
                    self.asg[(h, g)] = t

                def chunk(self, ci, h):
                    g, s = ci // self.grp, ci % self.grp
                    self._ensure(h, g)
                    return self.asg[(h, g)][:, s, :], self.msgs[(h, g)][:, s, :]

            def agg_window(plan, w, psum_shape, tag):
                """Accumulate one window's chunks into a PSUM tile."""
                cpw, base = plan.struct["cpw"], plan.struct["base"]
                ps = pagg.tile(psum_shape, f32, tag=tag)
                total = int(cpw[w, 0] + cpw[w, 1])
                k = 0
                for h in (0, 1):
                    for j in range(int(cpw[w, h])):
                        ci = int(base[w, h]) + j
                        asg_ap, msg_ap = plan.chunk(ci, h)
                        nc.tensor.matmul(
                            out=ps[:],
                            lhsT=asg_ap,
                            rhs=msg_ap,
                            start=(k == 0),
                            stop=(k == total - 1),
                        )
                        k += 1
                return ps

            WCH = {"Wr1T": wt["Wr1T"], "Wl1T": wt["Wl1T"],
                   "Wr2T": wt["Wr2T"], "Wl2T": wt["Wl2T"]}

            def wslice(nm, k):
                return WCH[nm][:, k * GDIM : (k + 1) * GDIM]

            def ag_chunk(src_list, dst_full, q):
                nc.gpsimd.collective_compute(
                    "AllGather",
                    mybir.AluOpType.bypass,
                    replica_groups=RG,
                    ins=[src_list[q].ap().opt()],
                    outs=[dst_full.ap()[q * QCHUNK * NCORES : (q + 1) * QCHUNK * NCORES, :].opt()],
                )

            # ================= Phase L: lap agg + r1 dense + AG1 ==========
            with nc.named_scope("lap"):
                lap_plan = AggPlan(
                    lap_struct, lap_idx_t,
                    (x16.ap()[0:T0_END, :], x16.ap()[T1_OFF:N, :]),
                    D, GRP, mlap, lap_dstl_t, lap_w_t,
                )

                r1_loads = {}

                def issue_r1_loads(b):
                    q, r0 = b // 3, (b % 3) * 512
                    lapT, xT = [], []
                    for k in (0, 1):
                        t = tpool.tile([128, 512], fp16, tag="tT")
                        nc.sync.dma_start_transpose(
                            out=t[:],
                            in_=xcq[q][r0 : r0 + 512,
                                       D + k * 128 : D + (k + 1) * 128],
                        )
                        lapT.append(t)
                    for k in (0, 1):
                        t = tpool.tile([128, 512], fp16, tag="tT")
                        nc.sync.dma_start(
                            out=t[:],
                            in_=xT16[k * 128 : (k + 1) * 128, b * 512 : (b + 1) * 512],
                        )
                        xT.append(t)
                    r1_loads[b] = (xT, lapT)

                def r1_dense(b):
                    xT, lapT = r1_loads.pop(b)
                    for nt in range(4):
                        nsl = slice(nt * 128, (nt + 1) * 128)
                        ps = pbig.tile([128, GDIM], f32, tag="pbig")
                        mms = [(xT[0], 0), (xT[1], 1), (lapT[0], 2), (lapT[1], 3)]
                        for i, (lt, k) in enumerate(mms):
                            nc.tensor.matmul(
                                out=ps[:], lhsT=lt[:, nsl], rhs=wslice("Wr1T", k),
                                start=(i == 0), stop=False,
                            )
                        nc.tensor.matmul(
                            out=ps[:], lhsT=ones1[:], rhs=b1t[:],
                            start=False, stop=True,
                        )
                        nc.vector.tensor_copy(r_sbuf[:, b * 4 + nt, :], ps[:])

                for w in range(NWIN):
                    ps = agg_window(lap_plan, w, [128, D], "plap")
                    lt = opool.tile([128, D], fp16, tag="lt")
                    nc.vector.tensor_tensor(
                        out=lt[:],
                        in0=ps[:],
                        in1=xw_all[:, w * D : (w + 1) * D],
                        op=mybir.AluOpType.add,
                    )
                    nc.sync.dma_start(
                        out=xcq[w // 12].ap()[(w % 12) * 128 : (w % 12 + 1) * 128,
                                              D : 2 * D],
                        in_=lt[:],
                    )
                    if phases >= 2 and w % 12 == 11:
                        ag_chunk(xcq, xcomb_full, w // 12)
                    if w % 4 == 0 and w >= 4:
                        issue_r1_loads(w // 4 - 1)
                    if w % 4 == 3 and w >= 7:
                        r1_dense(w // 4 - 1)
                issue_r1_loads(NBLK - 1)
                r1_dense(NBLK - 1)

            # ================= Phase C1: conv1 agg + dense + r2 + AG2 =====
            if phases >= 3:
                with nc.named_scope("conv1"):
                    c1_plan = AggPlan(
                        conv_struct, conv_idx_t,
                        (xcomb_full.ap()[0:T0_END, :], xcomb_full.ap()[T1_OFF:N, :]),
                        2 * D, GRP, mconv, conv_dstl_t, None,
                    )

                    mT_loads = {}
                    hT_loads = {}

                    def issue_mT_loads(b, dram, store, quartered=False):
                        if quartered:
                            dram, r0 = dram[b // 3], (b % 3) * 512
                        else:
                            r0 = b * 512
                        tiles = []
                        for k in range(4):
                            t = tpool.tile([128, 512], fp16, tag="tT")
                            nc.sync.dma_start_transpose(
                                out=t[:],
                                in_=dram[r0 : r0 + 512, k * 128 : (k + 1) * 128],
                            )
                            tiles.append(t)
                        store[b] = tiles

                    def c1_window(w):
                        ps = agg_window(c1_plan, w, [128, GDIM], "pagg")
                        mt = opool.tile([128, GDIM], fp16, tag="o16")
                        nc.vector.tensor_tensor(
                            out=mt[:],
                            in0=ps[:],
                            in1=cinv_t[:, w : w + 1].to_broadcast([128, GDIM]),
                            op=mybir.AluOpType.mult,
                        )
                        nc.sync.dma_start(
                            out=m1_dram[w * 128 : (w + 1) * 128, :], in_=mt[:]
                        )

                    def l1_dense(b):
                        mT = mT_loads.pop(b)
                        for nt in range(4):
                            nsl = slice(nt * 128, (nt + 1) * 128)
                            ps = pbig.tile([128, GDIM], f32, tag="pbig")
                            for k in range(4):
                                nc.tensor.matmul(
                                    out=ps[:], lhsT=mT[k][:, nsl],
                                    rhs=wslice("Wl1T", k),
                                    start=(k == 0), stop=(k == 3),
                                )
                            gt = gpool.tile([128, GDIM], fp16, tag="gl")
                            nc.vector.tensor_tensor(
                                out=gt[:], in0=ps[:],
                                in1=r_sbuf[:, b * 4 + nt, :],
                                op=mybir.AluOpType.add,
                            )
                            ht = opool.tile([128, GDIM], fp16, tag="o16")
                            nc.scalar.activation(
                                ht[:], gt[:], mybir.ActivationFunctionType.Gelu
                            )
                            r0 = (b % 3) * 512 + nt * 128
                            nc.sync.dma_start(
                                out=hq[b // 3][r0 : r0 + 128, :], in_=ht[:]
                            )

                    def r2_dense(b):
                        hT = hT_loads.pop(b)
                        for nt in range(4):
                            nsl = slice(nt * 128, (nt + 1) * 128)
                            ps = pbig.tile([128, GDIM], f32, tag="pbig")
                            for k in range(4):
                                nc.tensor.matmul(
                                    out=ps[:], lhsT=hT[k][:, nsl],
                                    rhs=wslice("Wr2T", k),
                                    start=(k == 0), stop=False,
                                )
                            nc.tensor.matmul(
                                out=ps[:], lhsT=ones1[:], rhs=b2t[:],
                                start=False, stop=True,
                            )
                            nc.vector.tensor_copy(r_sbuf[:, b * 4 + nt, :], ps[:])

                    for step in range(NBLK + 2):
                        if step < NBLK:
                            for j in range(4):
                                c1_window(step * 4 + j)
                        if 1 <= step <= NBLK:
                            l1_dense(step - 1)
                        if 2 <= step <= NBLK + 1:
                            r2_dense(step - 2)
                        if step < NBLK:
                            issue_mT_loads(step, m1_dram, mT_loads)
                        if 1 <= step <= NBLK:
                            issue_mT_loads(step - 1, hq, hT_loads, quartered=True)
                        if phases >= 4 and step in (3, 6, 9, 12):
                            ag_chunk(hq, h_full, (step - 3) // 3)

            # ================= Phase C2: conv2 agg + dense + pool =========
            if phases >= 5:
                with nc.named_scope("conv2"):
                    c2_plan = AggPlan(
                        conv_struct, conv_idx_t,
                        (h_full.ap()[0:T0_END, :], h_full.ap()[T1_OFF:N, :]),
                        GDIM, GRP, mconv, conv_dstl_t, None,
                    )

                    m2T_loads = {}

                    def issue_m2T_loads(b):
                        tiles = []
                        for k in range(4):
                            t = tpool.tile([128, 512], fp16, tag="tT")
                            nc.sync.dma_start_transpose(
                                out=t[:],
                                in_=m2_dram[b * 512 : (b + 1) * 512,
                                            k * 128 : (k + 1) * 128],
                            )
                            tiles.append(t)
                        m2T_loads[b] = tiles

                    def c2_window(w):
                        ps = agg_window(c2_plan, w, [128, GDIM], "pagg")
                        mt = opool.tile([128, GDIM], fp16, tag="o16")
                        nc.vector.tensor_tensor(
                            out=mt[:],
                            in0=ps[:],
                            in1=cinv_t[:, w : w + 1].to_broadcast([128, GDIM]),
                            op=mybir.AluOpType.mult,
                        )
                        nc.sync.dma_start(
                            out=m2_dram[w * 128 : (w + 1) * 128, :], in_=mt[:]
                        )

                    ps_pool = ppool.tile([128, GDIM], f32, tag="ppool")

                    def l2_dense(b):
                        mT = m2T_loads.pop(b)
                        for nt in range(4):
                            nsl = slice(nt * 128, (nt + 1) * 128)
                            ps = pbig.tile([128, GDIM], f32, tag="pbig")
                            for k in range(4):
                                nc.tensor.matmul(
                                    out=ps[:], lhsT=mT[k][:, nsl],
                                    rhs=wslice("Wl2T", k),
                                    start=(k == 0), stop=(k == 3),
                                )
                            ot = opool.tile([128, GDIM], fp16, tag="o16")
                            nc.vector.tensor_tensor(
                                out=ot[:], in0=ps[:],
                                in1=r_sbuf[:, b * 4 + nt, :],
                                op=mybir.AluOpType.add,
                            )
                            ntg = b * 4 + nt
                            nc.tensor.matmul(
                                out=ps_pool[:],
                                lhsT=pool_asg[:, ntg, :],
                                rhs=ot[:],
                                start=(ntg == 0),
                                stop=(ntg == NWIN - 1),
                            )

                    for step in range(NBLK + 1):
                        if step < NBLK:
                            for j in range(4):
                                c2_window(step * 4 + j)
                        if 1 <= step <= NBLK:
                            l2_dense(step - 1)
                        if step < NBLK:
                            issue_m2T_loads(step)

                    out_f = f32pool.tile([128, GDIM], f32, tag="of32")
                    nc.vector.tensor_scalar_mul(out_f[:], ps_pool[:], 1.0 / S)
                    nc.sync.dma_start(out=o_pool[:], in_=out_f[:])

            if phases < 5:
                dbg = f32pool.tile([128, GDIM], f32, tag="of32")
                nc.gpsimd.memset(dbg[:], 0.0)
                nc.sync.dma_start(out=o_pool[:], in_=dbg[:])
            if kdump:
                for q in range(4):
                    nc.sync.dma_start(
                        out=dumps["o_xc"].ap()[q * QCHUNK : (q + 1) * QCHUNK, :],
                        in_=xcq[q].ap(),
                    )
                    nc.sync.dma_start(
                        out=dumps["o_h"].ap()[q * QCHUNK : (q + 1) * QCHUNK, :],
                        in_=hq[q].ap(),
                    )
                for nm, dram in (("o_m1", m1_dram), ("o_m2", m2_dram)):
                    nc.sync.dma_start(out=dumps[nm][:], in_=dram[:])
                nc.sync.dma_start(
                    out=dumps["o_r"][:],
                    in_=r_sbuf[:].reshape([128, NWIN * GDIM]),
                )

    nc.finalize()
    return nc


LAST_EXEC_NS = None
LAST_SCOPES = None


def _maybe_install_trace_hook():
    """Optional NTFF profiling (KTRACE=1): register the axon profile hook."""
    import sys
    import types

    try:
        from trn_agent_boot.trn_boot import _ntff_profile_via_ctypes

        hook = _ntff_profile_via_ctypes("/opt/axon/libaxon_pjrt.so")
        mod = types.ModuleType("antenv.axon_hooks")
        mod.get_axon_ntff_profile_hook = lambda: hook
        mod.set_axon_ntff_profile_hook = lambda h: None
        sys.modules["antenv.axon_hooks"] = mod
        return True
    except Exception:
        return False


def _pack_w(W):
    """[GDIM, GDIM] contraction-major -> [128, 4*GDIM] SBUF-wrapped fp16."""
    WT = np.ascontiguousarray(W.T).astype(np.float16)         # [in, out]
    return np.ascontiguousarray(
        WT.reshape(4, 128, GDIM).transpose(1, 0, 2).reshape(128, 4 * GDIM)
    )


def kernel(**inputs):
    import os

    from concourse.bass_utils import run_bass_kernel_spmd

    x = np.asarray(inputs["sub2gene_out"], np.float32).reshape(N, D)
    edge_index = np.asarray(inputs["edge_index"])
    W_l1 = np.asarray(inputs["W_l1"], np.float32)
    W_r1 = np.asarray(inputs["W_r1"], np.float32)
    b1 = np.asarray(inputs["b1"], np.float32)
    W_l2 = np.asarray(inputs["W_l2"], np.float32)
    W_r2 = np.asarray(inputs["W_r2"], np.float32)
    b2 = np.asarray(inputs["b2"], np.float32)

    prep = _host_prep(edge_index)
    lap_struct, lap_pc = prep["lap"]
    conv_struct, conv_pc = prep["conv"]

    nc = _build_program(lap_struct, conv_struct)

    x16 = x.astype(np.float16)
    wts = {
        "Wr1T": _pack_w(W_r1),
        "Wl1T": _pack_w(W_l1),
        "Wr2T": _pack_w(W_r2),
        "Wl2T": _pack_w(W_l2),
    }
    in_maps = []
    for c in range(NCORES):
        xo = x16[c * NLOC : (c + 1) * NLOC]
        m = {
            "x16": x16,
            "x16own": xo,
            "xT16": np.ascontiguousarray(xo.T),
            "xwrap": np.ascontiguousarray(
                xo.reshape(NWIN, 128, D).transpose(1, 0, 2).reshape(128, NWIN * D)
            ),
            "iota": prep["iota"],
            "ones1": np.ones((1, 128), np.float16),
            "cinv": prep["cinv"][c],
            "pool_dstl": prep["pool_dstl"],
            "b1T": b1.astype(np.float16)[None, :],
            "b2T": b2.astype(np.float16)[None, :],
            **wts,
        }
        for h in (0, 1):
            m[f"lap_idx{h}"] = lap_pc[c]["idx"][h]
            m[f"lap_dstl{h}"] = lap_pc[c]["dstl"][h]
            m[f"lap_w{h}"] = lap_pc[c]["wgt"][h]
            m[f"conv_idx{h}"] = conv_pc[c]["idx"][h]
            m[f"conv_dstl{h}"] = conv_pc[c]["dstl"][h]
        in_maps.append(m)

    trace = os.environ.get("KTRACE") == "1" and _maybe_install_trace_hook()
    res = run_bass_kernel_spmd(nc, in_maps, core_ids=list(range(NCORES)), trace=trace)
    global LAST_EXEC_NS, LAST_SCOPES, LAST_RESULTS, LAST_RES
    LAST_EXEC_NS = res.exec_time_ns
    LAST_SCOPES = res.per_core_scope_times
    LAST_RESULTS = res.results
    LAST_RES = res
    out = np.concatenate([res.results[c]["o_pool"] for c in range(NCORES)], axis=0)
    return out.astype(np.float32)


# revision 16
# speedup vs baseline: 1.1496x; 1.1496x over previous
"""DrugGraphEmbedding (2x SAGEConv + sym-Laplacian features + mean-pool) on 8 trn2 cores.

Strategy: node-shard the 1024 graphs (128 graphs = 6144 nodes per core).
Aggregations (Laplacian smoothing, SAGE mean over incoming edges) run as
dma_gather of source rows + one-hot PE matmuls that scatter 128-edge chunks
into 128-destination PSUM windows.

v3 structure (vs original baseline):
  - Gather idx streams use two OVERLAPPING int16 tables [0,32K) and
    [16K,48K) with per-(core,window) balanced split -> fewer padded chunks.
  - Gather calls batch 8 chunks (1024 idxs) to amortize SWDGE fixed cost.
  - The two AllGathers block the gpsimd queue and are serialized against
    DMA-transposes by the Tile framework, so they are issued at phase ends
    and OVERLAPPED with dense root-term (x@Wr) compute that uses only plain
    DMA loads + TensorE transposes:
      AG1 || r1 = xcomb_own @ Wr1^T + b1   (lap features kept in SBUF)
      AG2 || r2 = h_own @ Wr2^T + b2       (h reloaded node-major)
  - l-terms (m @ Wl) + GELU + pool are software-pipelined INTO the
    gather/aggregate loops of the next phase (m1T/m2T via DMA-transpose,
    legal there since no collective is in flight).
  - Constants are host-packed (contiguous loads); idx tables load first on
    the sync queue, bulk constants on the scalar queue.
Everything computes in fp16 with f32 PSUM accumulation.
"""

import numpy as np

B, S, D = 1024, 48, 256
GDIM = 512
N = B * S            # 49152
E = 4 * N            # 196608
NCORES = 8
NLOC = N // NCORES   # 6144
WIN = 128            # dst nodes per PSUM window
NWIN = NLOC // WIN   # 48
NBLK = NWIN // 4     # 12 (512-node dense blocks)
T0_END = 32768       # gather table 0 = rows [0, 32768)
T1_OFF = 16384       # gather table 1 = rows [16384, 49152)


def _cdiv(a, b):
    return -(-a // b)


def _pack_idx(idx_stream):
    """int16 stream -> [128, len/16] wrapped tile (16 partitions, replicated x8)."""
    L = len(idx_stream)
    assert L % 16 == 0
    w = idx_stream.reshape(L // 16, 16).T  # [16, L/16]
    return np.tile(w, (8, 1)).astype(np.int16)


def _build_streams(dst, src, wgt):
    """Pad edges into per-(core, window, table) chunk streams with a shared
    chunks-per-window structure (SPMD: same program on every core).

    Tables OVERLAP: table0 = rows [0, T0_END), table1 = rows [T1_OFF, N).
    Edges with src in the overlap region [T1_OFF, T0_END) are assigned
    per-(core, window) to balance chunk counts (minimizes padding)."""
    gwin = dst // WIN                    # [E'] global window id 0..NCORES*NWIN-1
    cat = np.where(src < T1_OFF, 0, np.where(src < T0_END, 1, 2))  # lo/mid/hi
    order = np.lexsort((src, cat, gwin))
    dst_s, src_s = dst[order], src[order]
    wgt_s = wgt[order] if wgt is not None else None

    NW = NCORES * NWIN
    cnt = np.bincount(gwin, minlength=NW).reshape(NCORES, NWIN)
    cntlo = np.bincount(gwin[cat == 0], minlength=NW).reshape(NCORES, NWIN)
    cnthi = np.bincount(gwin[cat == 2], minlength=NW).reshape(NCORES, NWIN)

    K = np.maximum(_cdiv(cnt, 128).max(axis=0), 2)        # [NWIN] chunks/window
    k0 = np.maximum(_cdiv(cntlo, 128).max(axis=0), 1)     # table0 chunks
    k1 = np.maximum(K - k0, np.maximum(_cdiv(cnthi, 128).max(axis=0), 1))

    nch = [int(k0.sum()), int(k1.sum())]
    base = np.zeros((NWIN, 2), np.int64)
    base[1:, 0] = np.cumsum(k0[:-1])
    base[1:, 1] = np.cumsum(k1[:-1])
    cpw = np.stack([k0, k1], axis=1)

    flat_cnt = cnt.reshape(-1)
    starts = np.concatenate([[0], np.cumsum(flat_cnt)[:-1]]).reshape(NCORES, NWIN)
    cnt_nonhi = cnt - cnthi

    per_core = []
    for c in range(NCORES):
        idx_h = [np.zeros(nch[h] * 128, np.int64) for h in (0, 1)]
        dl_h = [np.full(nch[h] * 128, -1.0, np.float16) for h in (0, 1)]
        wg_h = [np.zeros(nch[h] * 128, np.float16) for h in (0, 1)]
        for w in range(NWIN):
            s0 = int(starts[c, w])
            n = int(cnt[c, w])
            a = min(128 * int(k0[w]), int(cnt_nonhi[c, w]))
            sw_src = src_s[s0 : s0 + n]
            sw_dst = dst_s[s0 : s0 + n]
            sw_w = wgt_s[s0 : s0 + n] if wgt_s is not None else None
            for h, (i0, i1) in enumerate(((0, a), (a, n))):
                m = i1 - i0
                assert m <= 128 * int(cpw[w, h]), (c, w, h, m, cpw[w, h])
                p0 = int(base[w, h]) * 128
                ss = sw_src[i0:i1] - (T1_OFF if h else 0)
                if m:
                    assert ss.min() >= 0 and ss.max() < 32768
                idx_h[h][p0 : p0 + m] = ss
                dl_h[h][p0 : p0 + m] = (sw_dst[i0:i1] % WIN).astype(np.float16)
                if sw_w is not None:
                    wg_h[h][p0 : p0 + m] = sw_w[i0:i1].astype(np.float16)
                else:
                    wg_h[h][p0 : p0 + m] = 1.0
        per_core.append({
            "idx": [_pack_idx(idx_h[h].astype(np.int16)) for h in (0, 1)],
            "dstl": [np.ascontiguousarray(dl_h[h].reshape(nch[h], 128).T) for h in (0, 1)],
            "wgt": [np.ascontiguousarray(wg_h[h].reshape(nch[h], 128).T) for h in (0, 1)],
        })

    return {"cpw": cpw, "base": base, "nch": nch}, per_core


def _host_prep(edge_index):
    row = np.asarray(edge_index[0], np.int64)
    col = np.asarray(edge_index[1], np.int64)

    deg = np.bincount(row, minlength=N).astype(np.float64)
    dinv = (deg > 0) / np.sqrt(np.maximum(deg, 1.0))
    cnt = np.bincount(col, minlength=N).astype(np.float64)
    cinv = 1.0 / np.maximum(cnt, 1.0)

    lap_w = -(dinv[row] * dinv[col])
    lap_struct, lap_pc = _build_streams(row, col, lap_w)     # dst=row, src=col
    conv_struct, conv_pc = _build_streams(col, row, None)    # dst=col, src=row

    cinv_tiles = []
    for c in range(NCORES):
        ci = cinv[c * NLOC : (c + 1) * NLOC].reshape(NWIN, WIN).T  # [128, NWIN]
        cinv_tiles.append(np.ascontiguousarray(ci).astype(np.float32))

    pool_dstl = np.zeros((128, NWIN), np.float16)
    for nt in range(NWIN):
        pool_dstl[:, nt] = ((nt * 128 + np.arange(128)) // S).astype(np.float16)

    iota = np.tile(np.arange(128, dtype=np.float16)[None, :], (128, 1))
    return {
        "lap": (lap_struct, lap_pc),
        "conv": (conv_struct, conv_pc),
        "cinv": cinv_tiles,
        "pool_dstl": pool_dstl,
        "iota": iota,
    }


def _build_program(lap_struct, conv_struct):
    import os

    import concourse.bass as bass
    import concourse.bacc as bacc
    import concourse.mybir as mybir
    from concourse.tile import TileContext

    phases = int(os.environ.get("KPHASES", "5"))
    GRP = int(os.environ.get("KGRP", "8"))
    single_packet = os.environ.get("KSP", "0") == "1"

    fp16 = mybir.dt.float16
    f32 = mybir.dt.float32
    i16 = mybir.dt.int16

    nc = bacc.Bacc(
        "TRN2",
        target_bir_lowering=False,
        debug=False,
        num_devices=NCORES,
        dynamic_dma_scratch_size=24576,
        num_swdge_queues=4,
    )

    # ---- inputs -----------------------------------------------------------
    x16 = nc.dram_tensor("x16", [N, D], fp16, kind="ExternalInput")
    x16own = nc.dram_tensor("x16own", [NLOC, D], fp16, kind="ExternalInput")
    xT16 = nc.dram_tensor("xT16", [D, NLOC], fp16, kind="ExternalInput")
    xw_in = nc.dram_tensor("xwrap", [128, NWIN * D], fp16, kind="ExternalInput")
    iota_in = nc.dram_tensor("iota", [128, 128], fp16, kind="ExternalInput")
    ident_in = nc.dram_tensor("ident", [128, 128], fp16, kind="ExternalInput")
    ones_in = nc.dram_tensor("ones1", [1, 128], fp16, kind="ExternalInput")
    cinv_in = nc.dram_tensor("cinv", [128, NWIN], f32, kind="ExternalInput")
    pdstl_in = nc.dram_tensor("pool_dstl", [128, NWIN], fp16, kind="ExternalInput")

    wts_in = {}
    for nm in ("Wr1T", "Wl1T", "Wr2T", "Wl2T"):
        wts_in[nm] = nc.dram_tensor(nm, [128, 4 * GDIM], fp16, kind="ExternalInput")
    b1_in = nc.dram_tensor("b1T", [1, GDIM], fp16, kind="ExternalInput")
    b2_in = nc.dram_tensor("b2T", [1, GDIM], fp16, kind="ExternalInput")

    lap_nch, conv_nch = lap_struct["nch"], conv_struct["nch"]
    lap_idx_in, conv_idx_in, lap_dstl_in, lap_w_in, conv_dstl_in = [], [], [], [], []
    for h in (0, 1):
        lap_idx_in.append(
            nc.dram_tensor(f"lap_idx{h}", [128, lap_nch[h] * 8], i16, kind="ExternalInput")
        )
        conv_idx_in.append(
            nc.dram_tensor(f"conv_idx{h}", [128, conv_nch[h] * 8], i16, kind="ExternalInput")
        )
        lap_dstl_in.append(
            nc.dram_tensor(f"lap_dstl{h}", [128, lap_nch[h]], fp16, kind="ExternalInput")
        )
        lap_w_in.append(
            nc.dram_tensor(f"lap_w{h}", [128, lap_nch[h]], fp16, kind="ExternalInput")
        )
        conv_dstl_in.append(
            nc.dram_tensor(f"conv_dstl{h}", [128, conv_nch[h]], fp16, kind="ExternalInput")
        )

    o_pool = nc.dram_tensor("o_pool", [128, GDIM], f32, kind="ExternalOutput")
    kdump = os.environ.get("KDUMP") == "1"
    dumps = {}
    if kdump:
        dumps["o_xc"] = nc.dram_tensor("o_xc", [NLOC, 2 * D], fp16, kind="ExternalOutput")
        dumps["o_m1"] = nc.dram_tensor("o_m1", [NLOC, GDIM], fp16, kind="ExternalOutput")
        dumps["o_h"] = nc.dram_tensor("o_h", [NLOC, GDIM], fp16, kind="ExternalOutput")
        dumps["o_m2"] = nc.dram_tensor("o_m2", [NLOC, GDIM], fp16, kind="ExternalOutput")
        dumps["o_r"] = nc.dram_tensor("o_r", [128, NWIN * GDIM], fp16, kind="ExternalOutput")

    # ---- internal DRAM ----------------------------------------------------
    xcomb_own = nc.dram_tensor("xcomb_own", [NLOC, 2 * D], fp16)
    xcomb_full = nc.dram_tensor("xcomb_full", [N, 2 * D], fp16, addr_space="Shared")
    h_own = nc.dram_tensor("h_own", [NLOC, GDIM], fp16)
    h_full = nc.dram_tensor("h_full", [N, GDIM], fp16, addr_space="Shared")
    m1_dram = nc.dram_tensor("m1_dram", [NLOC, GDIM], fp16)
    m2_dram = nc.dram_tensor("m2_dram", [NLOC, GDIM], fp16)

    RG = [list(range(NCORES))]

    with TileContext(nc) as tc:
        with (
            tc.tile_pool(name="const", bufs=1) as cpool,
            tc.tile_pool(name="mlap", bufs=3) as mlap,
            tc.tile_pool(name="mconv", bufs=3) as mconv,
            tc.tile_pool(name="asg", bufs=4) as apool,
            tc.tile_pool(name="tT", bufs=8) as tpool,
            tc.tile_pool(name="lT", bufs=2) as lTpool,
            tc.tile_pool(name="o16", bufs=4) as opool,
            tc.tile_pool(name="gl", bufs=2) as gpool,
            tc.tile_pool(name="of32", bufs=1) as f32pool,
            tc.tile_pool(name="pagg", bufs=2, space="PSUM") as pagg,
            tc.tile_pool(name="pbig", bufs=2, space="PSUM") as pbig,
            tc.tile_pool(name="ptr", bufs=2, space="PSUM") as ptrp,
            tc.tile_pool(name="ppool", bufs=1, space="PSUM") as ppool,
        ):
            # ---- critical-path constants on sync queue -------------------
            iota = cpool.tile([128, 128], fp16, tag="iota")
            nc.sync.dma_start(out=iota[:], in_=iota_in[:])
            lap_idx_t, conv_idx_t, lap_dstl_t, lap_w_t, conv_dstl_t = [], [], [], [], []
            for h in (0, 1):
                t = cpool.tile([128, lap_nch[h] * 8], i16, tag=f"lidx{h}")
                nc.sync.dma_start(out=t[:], in_=lap_idx_in[h][:])
                lap_idx_t.append(t)
                t = cpool.tile([128, lap_nch[h]], fp16, tag=f"ldstl{h}")
                nc.sync.dma_start(out=t[:], in_=lap_dstl_in[h][:])
                lap_dstl_t.append(t)
                t = cpool.tile([128, lap_nch[h]], fp16, tag=f"lw{h}")
                nc.sync.dma_start(out=t[:], in_=lap_w_in[h][:])
                lap_w_t.append(t)
            for h in (0, 1):
                t = cpool.tile([128, conv_nch[h] * 8], i16, tag=f"cidx{h}")
                nc.sync.dma_start(out=t[:], in_=conv_idx_in[h][:])
                conv_idx_t.append(t)
                t = cpool.tile([128, conv_nch[h]], fp16, tag=f"cdstl{h}")
                nc.sync.dma_start(out=t[:], in_=conv_dstl_in[h][:])
                conv_dstl_t.append(t)

            # ---- bulk constants on scalar queue --------------------------
            ones1 = cpool.tile([1, 128], fp16, tag="ones1")
            nc.scalar.dma_start(out=ones1[:], in_=ones_in[:])
            ident = cpool.tile([128, 128], fp16, tag="ident")
            nc.scalar.dma_start(out=ident[:], in_=ident_in[:])
            cinv_t = cpool.tile([128, NWIN], f32, tag="cinv")
            nc.scalar.dma_start(out=cinv_t[:], in_=cinv_in[:])
            pdstl = cpool.tile([128, NWIN], fp16, tag="pdstl")
            nc.scalar.dma_start(out=pdstl[:], in_=pdstl_in[:])
            b1t = cpool.tile([1, GDIM], fp16, tag="b1")
            nc.scalar.dma_start(out=b1t[:], in_=b1_in[:])
            b2t = cpool.tile([1, GDIM], fp16, tag="b2")
            nc.scalar.dma_start(out=b2t[:], in_=b2_in[:])
            xw_all = cpool.tile([128, NWIN, D], fp16, tag="xw")
            nc.scalar.dma_start(out=xw_all[:], in_=xw_in[:])
            wt = {}
            for nm in ("Wr1T", "Wl1T", "Wr2T", "Wl2T"):
                t = cpool.tile([128, 4 * GDIM], fp16, tag=nm)
                nc.scalar.dma_start(out=t[:], in_=wts_in[nm][:])
                wt[nm] = t
            # x-half of xcomb (DRAM->DRAM, overlapped with lap gathers)
            nc.scalar.dma_start(out=xcomb_own.ap()[:, 0:D], in_=x16own.ap())

            # pool one-hot: [128, NWIN, 128]
            pool_asg = cpool.tile([128, NWIN, 128], fp16, tag="pasg")
            nc.vector.tensor_tensor(
                out=pool_asg[:],
                in0=pdstl[:].to_broadcast([128, NWIN, 128]),
                in1=iota[:, None, :].to_broadcast([128, NWIN, 128]),
                op=mybir.AluOpType.is_equal,
            )

            # persistent root-term buffer (r1 then overwritten in place by r2)
            r_sbuf = cpool.tile([128, NWIN, GDIM], fp16, tag="rterm")

            qctr = [0]

            class AggPlan:
                """Just-in-time gather + assign-build for one chunk-stream pass."""

                def __init__(self, struct, idx_tiles, tabs, elem, grp, pool,
                             dstl_tiles, w_tiles):
                    self.struct = struct
                    self.idx_tiles = idx_tiles
                    self.tabs = tabs            # (table0_ap, table1_ap)
                    self.elem = elem
                    self.grp = grp
                    self.pool = pool
                    self.dstl_tiles = dstl_tiles
                    self.w_tiles = w_tiles
                    self.msgs = {}
                    self.asg = {}

                def _ensure(self, h, g):
                    if (h, g) in self.msgs:
                        return
                    c0 = g * self.grp
                    cn = min(self.grp, self.struct["nch"][h] - c0)
                    ni = cn * 128
                    tile = self.pool.tile([128, self.grp, self.elem], fp16, tag="msgs")
                    nc.gpsimd.dma_gather(
                        out_ap=tile[:, 0:cn, :],
                        in_ap=self.tabs[h],
                        idxs_ap=self.idx_tiles[h][:, c0 * 8 : (c0 + cn) * 8],
                        num_idxs=ni,
                        num_idxs_reg=ni,
                        elem_size=self.elem,
                        single_packet=single_packet,
                        queue_num=qctr[0] % 4,
                    )
                    qctr[0] += 1
                    self.msgs[(h, g)] = tile
                    t = apool.tile([128, self.grp, 128], fp16, tag="asg")
                    nc.vector.tensor_tensor(
                        out=t[:, 0:cn, :],
                        in0=self.dstl_tiles[h][:, c0 : c0 + cn].to_broadcast(
                            [128, cn, 128]
                        ),
                        in1=iota[:, None, :].to_broadcast([128, cn, 128]),
                        op=mybir.AluOpType.is_equal,
                    )
                    if self.w_tiles is not None:
                        nc.vector.tensor_tensor(
                            out=t[:, 0:cn, :],
                            in0=t[:, 0:cn, :],
                            in1=self.w_tiles[h][:, c0 : c0 + cn].to_broadcast(
                                [128, cn, 128]
                            ),
                            op=mybir.AluOpType.mult,
                        )
                    self.asg[(h, g)] = t

                def chunk(self, ci, h):
                    g, s = ci // self.grp, ci % self.grp
                    self._ensure(h, g)
                    return self.asg[(h, g)][:, s, :], self.msgs[(h, g)][:, s, :]

            def agg_window(plan, w, fdim, tag):
                """Accumulate one window's chunks into a PSUM tile region."""
                cpw, base = plan.struct["cpw"], plan.struct["base"]
                ps = pagg.tile([128, GDIM], f32, tag=tag)
                total = int(cpw[w, 0] + cpw[w, 1])
                k = 0
                for h in (0, 1):
                    for j in range(int(cpw[w, h])):
                        ci = int(base[w, h]) + j
                        asg_ap, msg_ap = plan.chunk(ci, h)
                        nc.tensor.matmul(
                            out=ps[:, 0:fdim],
                            lhsT=asg_ap,
                            rhs=msg_ap,
                            start=(k == 0),
                            stop=(k == total - 1),
                        )
                        k += 1
                return ps

            def wslice(nm, k):
                return wt[nm][:, k * GDIM : (k + 1) * GDIM]

            # ================= Phase L: lap agg ===========================
            with nc.named_scope("lap"):
                lap_plan = AggPlan(
                    lap_struct, lap_idx_t,
                    (x16.ap()[0:T0_END, :], x16.ap()[T1_OFF:N, :]),
                    D, GRP, mlap, lap_dstl_t, lap_w_t,
                )
                for w in range(NWIN):
                    ps = agg_window(lap_plan, w, D, "pagg")
                    # lap_x = L_off@x + x, accumulated in place over the SBUF
                    # copy of x (retained for the TensorE transposes in r1)
                    nc.vector.tensor_tensor(
                        out=xw_all[:, w, :],
                        in0=ps[:, 0:D],
                        in1=xw_all[:, w, :],
                        op=mybir.AluOpType.add,
                    )
                    nc.sync.dma_start(
                        out=xcomb_own.ap()[w * 128 : (w + 1) * 128, D : 2 * D],
                        in_=xw_all[:, w, :],
                    )

            # ================= AG1 || r1 dense ============================
            if phases >= 2:
                nc.gpsimd.collective_compute(
                    "AllGather",
                    mybir.AluOpType.bypass,
                    replica_groups=RG,
                    ins=[xcomb_own.ap().opt()],
                    outs=[xcomb_full.ap().opt()],
                )

            with nc.named_scope("r1"):
                xT_loads = {}

                def issue_xT_loads(b):
                    xT = []
                    for k in (0, 1):
                        t = tpool.tile([128, 512], fp16, tag="tT")
                        nc.scalar.dma_start(
                            out=t[:],
                            in_=xT16[k * 128 : (k + 1) * 128, b * 512 : (b + 1) * 512],
                        )
                        xT.append(t)
                    xT_loads[b] = xT

                def r1_block(b):
                    xT = xT_loads.pop(b)
                    # lapT via TensorE transpose of SBUF-resident lap features
                    lT = lTpool.tile([128, 2, 512], fp16, tag="lT")
                    for j in range(4):
                        for k in (0, 1):
                            pt = ptrp.tile([128, 128], fp16, tag="ptr")
                            nc.tensor.transpose(
                                pt[:],
                                xw_all[:, b * 4 + j, k * 128 : (k + 1) * 128],
                                ident[:],
                            )
                            nc.vector.tensor_copy(
                                lT[:, k, j * 128 : (j + 1) * 128], pt[:]
                            )
                    for nt in range(4):
                        nsl = slice(nt * 128, (nt + 1) * 128)
                        ps = pbig.tile([128, GDIM], f32, tag="pbig")
                        mms = [(xT[0][:, nsl], 0), (xT[1][:, nsl], 1),
                               (lT[:, 0, nsl], 2), (lT[:, 1, nsl], 3)]
                        for i, (lhsT, k) in enumerate(mms):
                            nc.tensor.matmul(
                                out=ps[:], lhsT=lhsT, rhs=wslice("Wr1T", k),
                                start=(i == 0), stop=False,
                            )
                        nc.tensor.matmul(
                            out=ps[:], lhsT=ones1[:], rhs=b1t[:],
                            start=False, stop=True,
                        )
                        nc.vector.tensor_copy(r_sbuf[:, b * 4 + nt, :], ps[:])

                issue_xT_loads(0)
                for b in range(NBLK):
                    if b + 1 < NBLK:
                        issue_xT_loads(b + 1)
                    r1_block(b)

            # ================= Phase C1: conv1 agg + l1 dense =============
            if phases >= 3:
                with nc.named_scope("conv1"):
                    c1_plan = AggPlan(
                        conv_struct, conv_idx_t,
                        (xcomb_full.ap()[0:T0_END, :], xcomb_full.ap()[T1_OFF:N, :]),
                        2 * D, GRP, mconv, conv_dstl_t, None,
                    )

                    mT_loads = {}

                    def issue_mT_loads(b, dram, store):
                        tiles = []
                        for k in range(4):
                            t = tpool.tile([128, 512], fp16, tag="tT")
                            nc.sync.dma_start_transpose(
                                out=t[:],
                                in_=dram[b * 512 : (b + 1) * 512,
                                         k * 128 : (k + 1) * 128],
                            )
                            tiles.append(t)
                        store[b] = tiles

                    def c1_window(w):
                        ps = agg_window(c1_plan, w, GDIM, "pagg")
                        mt = opool.tile([128, GDIM], fp16, tag="o16")
                        nc.vector.tensor_tensor(
                            out=mt[:],
                            in0=ps[:],
                            in1=cinv_t[:, w : w + 1].to_broadcast([128, GDIM]),
                            op=mybir.AluOpType.mult,
                        )
                        nc.sync.dma_start(
                            out=m1_dram[w * 128 : (w + 1) * 128, :], in_=mt[:]
                        )

                    def l1_dense(b):
                        mT = mT_loads.pop(b)
                        for nt in range(4):
                            nsl = slice(nt * 128, (nt + 1) * 128)
                            ps = pbig.tile([128, GDIM], f32, tag="pbig")
                            for k in range(4):
                                nc.tensor.matmul(
                                    out=ps[:], lhsT=mT[k][:, nsl],
                                    rhs=wslice("Wl1T", k),
                                    start=(k == 0), stop=(k == 3),
                                )
                            gt = gpool.tile([128, GDIM], fp16, tag="gl")
                            nc.vector.tensor_tensor(
                                out=gt[:], in0=ps[:],
                                in1=r_sbuf[:, b * 4 + nt, :],
                                op=mybir.AluOpType.add,
                            )
                            ht = opool.tile([128, GDIM], fp16, tag="o16")
                            nc.scalar.activation(
                                ht[:], gt[:], mybir.ActivationFunctionType.Gelu
                            )
                            nc.sync.dma_start(
                                out=h_own[b * 512 + nt * 128 : b * 512 + (nt + 1) * 128, :],
                                in_=ht[:],
                            )

                    for step in range(NBLK + 1):
                        if step < NBLK:
                            for j in range(4):
                                c1_window(step * 4 + j)
                        if 1 <= step <= NBLK:
                            l1_dense(step - 1)
                        if step < NBLK:
                            issue_mT_loads(step, m1_dram, mT_loads)

            # ================= AG2 || r2 dense ============================
            if phases >= 4:
                nc.gpsimd.collective_compute(
                    "AllGather",
                    mybir.AluOpType.bypass,
                    replica_groups=RG,
                    ins=[h_own.ap().opt()],
                    outs=[h_full.ap().opt()],
                )

            if phases >= 3:
                with nc.named_scope("r2"):
                    hN_loads = {}

                    def issue_hN_loads(b):
                        tiles = []
                        for j in range(4):
                            t = tpool.tile([128, 512], fp16, tag="tT")
                            nc.scalar.dma_start(
                                out=t[:],
                                in_=h_own[b * 512 + j * 128 : b * 512 + (j + 1) * 128, :],
                            )
                            tiles.append(t)
                        hN_loads[b] = tiles

                    def r2_block(b):
                        hN = hN_loads.pop(b)
                        hT = lTpool.tile([128, 4, 512], fp16, tag="hT")
                        for j in range(4):
                            for k in range(4):
                                pt = ptrp.tile([128, 128], fp16, tag="ptr")
                                nc.tensor.transpose(
                                    pt[:],
                                    hN[j][:, k * 128 : (k + 1) * 128],
                                    ident[:],
                                )
                                nc.vector.tensor_copy(
                                    hT[:, k, j * 128 : (j + 1) * 128], pt[:]
                                )
                        for nt in range(4):
                            nsl = slice(nt * 128, (nt + 1) * 128)
                            ps = pbig.tile([128, GDIM], f32, tag="pbig")
                            for k in range(4):
                                nc.tensor.matmul(
                                    out=ps[:], lhsT=hT[:, k, nsl],
                                    rhs=wslice("Wr2T", k),
                                    start=(k == 0), stop=False,
                                )
                            nc.tensor.matmul(
                                out=ps[:], lhsT=ones1[:], rhs=b2t[:],
                                start=False, stop=True,
                            )
                            nc.vector.tensor_copy(r_sbuf[:, b * 4 + nt, :], ps[:])

                    issue_hN_loads(0)
                    for b in range(NBLK):
                        if b + 1 < NBLK:
                            issue_hN_loads(b + 1)
                        r2_block(b)

            # ================= Phase C2: conv2 agg + dense + pool =========
            if phases >= 5:
                with nc.named_scope("conv2"):
                    c2_plan = AggPlan(
                        conv_struct, conv_idx_t,
                        (h_full.ap()[0:T0_END, :], h_full.ap()[T1_OFF:N, :]),
                        GDIM, GRP, mconv, conv_dstl_t, None,
                    )

                    m2T_loads = {}

                    def issue_m2T_loads(b):
                        tiles = []
                        for k in range(4):
                            t = tpool.tile([128, 512], fp16, tag="tT")
                            nc.sync.dma_start_transpose(
                                out=t[:],
                                in_=m2_dram[b * 512 : (b + 1) * 512,
                                            k * 128 : (k + 1) * 128],
                            )
                            tiles.append(t)
                        m2T_loads[b] = tiles

                    def c2_window(w):
                        ps = agg_window(c2_plan, w, GDIM, "pagg")
                        mt = opool.tile([128, GDIM], fp16, tag="o16")
                        nc.vector.tensor_tensor(
                            out=mt[:],
                            in0=ps[:],
                            in1=cinv_t[:, w : w + 1].to_broadcast([128, GDIM]),
                            op=mybir.AluOpType.mult,
                        )
                        nc.sync.dma_start(
                            out=m2_dram[w * 128 : (w + 1) * 128, :], in_=mt[:]
                        )

                    ps_pool = ppool.tile([128, GDIM], f32, tag="ppool")

                    def l2_dense(b):
                        mT = m2T_loads.pop(b)
                        for nt in range(4):
                            nsl = slice(nt * 128, (nt + 1) * 128)
                            ps = pbig.tile([128, GDIM], f32, tag="pbig")
                            for k in range(4):
                                nc.tensor.matmul(
                                    out=ps[:], lhsT=mT[k][:, nsl],
                                    rhs=wslice("Wl2T", k),
                                    start=(k == 0), stop=(k == 3),
                                )
                            ot = opool.tile([128, GDIM], fp16, tag="o16")
                            nc.vector.tensor_tensor(
                                out=ot[:], in0=ps[:],
                                in1=r_sbuf[:, b * 4 + nt, :],
                                op=mybir.AluOpType.add,
                            )
                            ntg = b * 4 + nt
                            nc.tensor.matmul(
                                out=ps_pool[:],
                                lhsT=pool_asg[:, ntg, :],
                                rhs=ot[:],
                                start=(ntg == 0),
                                stop=(ntg == NWIN - 1),
                            )

                    for step in range(NBLK + 1):
                        if step < NBLK:
                            for j in range(4):
                                c2_window(step * 4 + j)
                        if 1 <= step <= NBLK:
                            l2_dense(step - 1)
                        if step < NBLK:
                            issue_m2T_loads(step)

                    out_f = f32pool.tile([128, GDIM], f32, tag="of32")
                    nc.vector.tensor_scalar_mul(out_f[:], ps_pool[:], 1.0 / S)
                    nc.sync.dma_start(out=o_pool[:], in_=out_f[:])

            if phases < 5:
                dbg = f32pool.tile([128, GDIM], f32, tag="of32")
                nc.gpsimd.memset(dbg[:], 0.0)
                nc.sync.dma_start(out=o_pool[:], in_=dbg[:])
            if kdump:
                for nm, dram in (("o_xc", xcomb_own), ("o_m1", m1_dram),
                                 ("o_h", h_own), ("o_m2", m2_dram)):
                    nc.sync.dma_start(out=dumps[nm][:], in_=dram[:])
                nc.sync.dma_start(out=dumps["o_r"][:], in_=r_sbuf[:])

    nc.finalize()
    return nc


LAST_EXEC_NS = None
LAST_SCOPES = None


def _maybe_install_trace_hook():
    """Optional NTFF profiling (KTRACE=1): register the axon profile hook."""
    import sys
    import types

    try:
        from trn_agent_boot.trn_boot import _ntff_profile_via_ctypes

        hook = _ntff_profile_via_ctypes("/opt/axon/libaxon_pjrt.so")
        mod = types.ModuleType("antenv.axon_hooks")
        mod.get_axon_ntff_profile_hook = lambda: hook
        mod.set_axon_ntff_profile_hook = lambda h: None
        sys.modules["antenv.axon_hooks"] = mod
        return True
    except Exception:
        return False


def _pack_w(W):
    """[GDIM, GDIM] -> [128, 4*GDIM] SBUF-wrapped fp16 (contraction-major)."""
    WT = np.ascontiguousarray(W.T).astype(np.float16)         # [in, out]
    return np.ascontiguousarray(
        WT.reshape(4, 128, GDIM).transpose(1, 0, 2).reshape(128, 4 * GDIM)
    )


def kernel(**inputs):
    import os

    from concourse.bass_utils import run_bass_kernel_spmd

    x = np.asarray(inputs["sub2gene_out"], np.float32).reshape(N, D)
    edge_index = np.asarray(inputs["edge_index"])
    W_l1 = np.asarray(inputs["W_l1"], np.float32)
    W_r1 = np.asarray(inputs["W_r1"], np.float32)
    b1 = np.asarray(inputs["b1"], np.float32)
    W_l2 = np.asarray(inputs["W_l2"], np.float32)
    W_r2 = np.asarray(inputs["W_r2"], np.float32)
    b2 = np.asarray(inputs["b2"], np.float32)

    prep = _host_prep(edge_index)
    lap_struct, lap_pc = prep["lap"]
    conv_struct, conv_pc = prep["conv"]

    nc = _build_program(lap_struct, conv_struct)

    x16 = x.astype(np.float16)
    wts = {
        "Wr1T": _pack_w(W_r1),
        "Wl1T": _pack_w(W_l1),
        "Wr2T": _pack_w(W_r2),
        "Wl2T": _pack_w(W_l2),
    }
    in_maps = []
    for c in range(NCORES):
        xo = x16[c * NLOC : (c + 1) * NLOC]
        m = {
            "x16": x16,
            "x16own": xo,
            "xT16": np.ascontiguousarray(xo.T),
            "xwrap": np.ascontiguousarray(
                xo.reshape(NWIN, 128, D).transpose(1, 0, 2).reshape(128, NWIN * D)
            ),
            "iota": prep["iota"],
            "ident": np.eye(128, dtype=np.float16),
            "ones1": np.ones((1, 128), np.float16),
            "cinv": prep["cinv"][c],
            "pool_dstl": prep["pool_dstl"],
            "b1T": b1.astype(np.float16)[None, :],
            "b2T": b2.astype(np.float16)[None, :],
            **wts,
        }
        for h in (0, 1):
            m[f"lap_idx{h}"] = lap_pc[c]["idx"][h]
            m[f"lap_dstl{h}"] = lap_pc[c]["dstl"][h]
            m[f"lap_w{h}"] = lap_pc[c]["wgt"][h]
            m[f"conv_idx{h}"] = conv_pc[c]["idx"][h]
            m[f"conv_dstl{h}"] = conv_pc[c]["dstl"][h]
        in_maps.append(m)

    trace = os.environ.get("KTRACE") == "1" and _maybe_install_trace_hook()
    res = run_bass_kernel_spmd(nc, in_maps, core_ids=list(range(NCORES)), trace=trace)
    global LAST_EXEC_NS, LAST_SCOPES, LAST_RESULTS, LAST_RES
    LAST_EXEC_NS = res.exec_time_ns
    LAST_SCOPES = res.per_core_scope_times
    LAST_RESULTS = res.results
    LAST_RES = res
    out = np.concatenate([res.results[c]["o_pool"] for c in range(NCORES)], axis=0)
    return out.astype(np.float32)


# revision 20
# speedup vs baseline: 1.1506x; 1.0008x over previous
"""DrugGraphEmbedding (2x SAGEConv + sym-Laplacian features + mean-pool) on 8 trn2 cores.

Strategy: node-shard the 1024 graphs (128 graphs = 6144 nodes per core).
Aggregations (Laplacian smoothing, SAGE mean over incoming edges) run as
dma_gather of source rows + one-hot PE matmuls that scatter 128-edge chunks
into 128-destination PSUM windows.

v3 structure (vs original baseline):
  - Gather idx streams use two OVERLAPPING int16 tables [0,32K) and
    [16K,48K) with per-(core,window) balanced split -> fewer padded chunks.
  - Gather calls batch 8 chunks (1024 idxs) to amortize SWDGE fixed cost.
  - The two AllGathers block the gpsimd queue and are serialized against
    DMA-transposes by the Tile framework, so they are issued at phase ends
    and OVERLAPPED with dense root-term (x@Wr) compute that uses only plain
    DMA loads + TensorE transposes:
      AG1 || r1 = xcomb_own @ Wr1^T + b1   (lap features kept in SBUF)
      AG2 || r2 = h_own @ Wr2^T + b2       (h reloaded node-major)
  - l-terms (m @ Wl) + GELU + pool are software-pipelined INTO the
    gather/aggregate loops of the next phase (m1T/m2T via DMA-transpose,
    legal there since no collective is in flight).
  - Constants are host-packed (contiguous loads); idx tables load first on
    the sync queue, bulk constants on the scalar queue.
Everything computes in fp16 with f32 PSUM accumulation.
"""

import numpy as np

B, S, D = 1024, 48, 256
GDIM = 512
N = B * S            # 49152
E = 4 * N            # 196608
NCORES = 8
NLOC = N // NCORES   # 6144
WIN = 128            # dst nodes per PSUM window
NWIN = NLOC // WIN   # 48
NBLK = NWIN // 4     # 12 (512-node dense blocks)
T0_END = 32768       # gather table 0 = rows [0, 32768)
T1_OFF = 16384       # gather table 1 = rows [16384, 49152)


def _cdiv(a, b):
    return -(-a // b)


def _pack_idx(idx_stream):
    """int16 stream -> [128, len/16] wrapped tile (16 partitions, replicated x8)."""
    L = len(idx_stream)
    assert L % 16 == 0
    w = idx_stream.reshape(L // 16, 16).T  # [16, L/16]
    return np.tile(w, (8, 1)).astype(np.int16)


def _build_streams(dst, src, wgt):
    """Pad edges into per-(core, window, table) chunk streams with a shared
    chunks-per-window structure (SPMD: same program on every core).

    Tables OVERLAP: table0 = rows [0, T0_END), table1 = rows [T1_OFF, N).
    Edges with src in the overlap region [T1_OFF, T0_END) are assigned
    per-(core, window) to balance chunk counts (minimizes padding)."""
    gwin = dst // WIN                    # [E'] global window id 0..NCORES*NWIN-1
    cat = np.where(src < T1_OFF, 0, np.where(src < T0_END, 1, 2))  # lo/mid/hi
    order = np.lexsort((src, cat, gwin))
    dst_s, src_s = dst[order], src[order]
    wgt_s = wgt[order] if wgt is not None else None

    NW = NCORES * NWIN
    cnt = np.bincount(gwin, minlength=NW).reshape(NCORES, NWIN)
    cntlo = np.bincount(gwin[cat == 0], minlength=NW).reshape(NCORES, NWIN)
    cnthi = np.bincount(gwin[cat == 2], minlength=NW).reshape(NCORES, NWIN)

    K = np.maximum(_cdiv(cnt, 128).max(axis=0), 2)        # [NWIN] chunks/window
    k0 = np.maximum(_cdiv(cntlo, 128).max(axis=0), 1)     # table0 chunks
    k1 = np.maximum(K - k0, np.maximum(_cdiv(cnthi, 128).max(axis=0), 1))

    nch = [int(k0.sum()), int(k1.sum())]
    base = np.zeros((NWIN, 2), np.int64)
    base[1:, 0] = np.cumsum(k0[:-1])
    base[1:, 1] = np.cumsum(k1[:-1])
    cpw = np.stack([k0, k1], axis=1)

    flat_cnt = cnt.reshape(-1)
    starts = np.concatenate([[0], np.cumsum(flat_cnt)[:-1]]).reshape(NCORES, NWIN)
    cnt_nonhi = cnt - cnthi

    per_core = []
    for c in range(NCORES):
        idx_h = [np.zeros(nch[h] * 128, np.int64) for h in (0, 1)]
        dl_h = [np.full(nch[h] * 128, -1.0, np.float16) for h in (0, 1)]
        wg_h = [np.zeros(nch[h] * 128, np.float16) for h in (0, 1)]
        for w in range(NWIN):
            s0 = int(starts[c, w])
            n = int(cnt[c, w])
            a = min(128 * int(k0[w]), int(cnt_nonhi[c, w]))
            sw_src = src_s[s0 : s0 + n]
            sw_dst = dst_s[s0 : s0 + n]
            sw_w = wgt_s[s0 : s0 + n] if wgt_s is not None else None
            for h, (i0, i1) in enumerate(((0, a), (a, n))):
                m = i1 - i0
                assert m <= 128 * int(cpw[w, h]), (c, w, h, m, cpw[w, h])
                p0 = int(base[w, h]) * 128
                ss = sw_src[i0:i1] - (T1_OFF if h else 0)
                if m:
                    assert ss.min() >= 0 and ss.max() < 32768
                idx_h[h][p0 : p0 + m] = ss
                dl_h[h][p0 : p0 + m] = (sw_dst[i0:i1] % WIN).astype(np.float16)
                if sw_w is not None:
                    wg_h[h][p0 : p0 + m] = sw_w[i0:i1].astype(np.float16)
                else:
                    wg_h[h][p0 : p0 + m] = 1.0
        per_core.append({
            "idx": [_pack_idx(idx_h[h].astype(np.int16)) for h in (0, 1)],
            "dstl": [np.ascontiguousarray(dl_h[h].reshape(nch[h], 128).T) for h in (0, 1)],
            "wgt": [np.ascontiguousarray(wg_h[h].reshape(nch[h], 128).T) for h in (0, 1)],
        })

    return {"cpw": cpw, "base": base, "nch": nch}, per_core


def _host_prep(edge_index):
    row = np.asarray(edge_index[0], np.int64)
    col = np.asarray(edge_index[1], np.int64)

    deg = np.bincount(row, minlength=N).astype(np.float64)
    dinv = (deg > 0) / np.sqrt(np.maximum(deg, 1.0))
    cnt = np.bincount(col, minlength=N).astype(np.float64)
    cinv = 1.0 / np.maximum(cnt, 1.0)

    lap_w = -(dinv[row] * dinv[col])
    lap_struct, lap_pc = _build_streams(row, col, lap_w)     # dst=row, src=col
    conv_struct, conv_pc = _build_streams(col, row, None)    # dst=col, src=row

    cinv_tiles = []
    for c in range(NCORES):
        ci = cinv[c * NLOC : (c + 1) * NLOC].reshape(NWIN, WIN).T  # [128, NWIN]
        cinv_tiles.append(np.ascontiguousarray(ci).astype(np.float32))

    pool_dstl = np.zeros((128, NWIN), np.float16)
    for nt in range(NWIN):
        pool_dstl[:, nt] = ((nt * 128 + np.arange(128)) // S).astype(np.float16)

    iota = np.tile(np.arange(128, dtype=np.float16)[None, :], (128, 1))
    return {
        "lap": (lap_struct, lap_pc),
        "conv": (conv_struct, conv_pc),
        "cinv": cinv_tiles,
        "pool_dstl": pool_dstl,
        "iota": iota,
    }


def _build_program(lap_struct, conv_struct):
    import os

    import concourse.bass as bass
    import concourse.bacc as bacc
    import concourse.mybir as mybir
    from concourse.tile import TileContext

    phases = int(os.environ.get("KPHASES", "5"))
    GRP = int(os.environ.get("KGRP", "8"))
    single_packet = os.environ.get("KSP", "0") == "1"

    fp16 = mybir.dt.float16
    f32 = mybir.dt.float32
    i16 = mybir.dt.int16

    nc = bacc.Bacc(
        "TRN2",
        target_bir_lowering=False,
        debug=False,
        num_devices=NCORES,
        dynamic_dma_scratch_size=24576,
        num_swdge_queues=4,
    )

    # ---- inputs -----------------------------------------------------------
    x16 = nc.dram_tensor("x16", [N, D], fp16, kind="ExternalInput")
    x16own = nc.dram_tensor("x16own", [NLOC, D], fp16, kind="ExternalInput")
    xT16 = nc.dram_tensor("xT16", [D, NLOC], fp16, kind="ExternalInput")
    xw_in = nc.dram_tensor("xwrap", [128, NWIN * D], fp16, kind="ExternalInput")
    iota_in = nc.dram_tensor("iota", [128, 128], fp16, kind="ExternalInput")
    ident_in = nc.dram_tensor("ident", [128, 128], fp16, kind="ExternalInput")
    ones_in = nc.dram_tensor("ones1", [1, 128], fp16, kind="ExternalInput")
    cinv_in = nc.dram_tensor("cinv", [128, NWIN], f32, kind="ExternalInput")
    pdstl_in = nc.dram_tensor("pool_dstl", [128, NWIN], fp16, kind="ExternalInput")

    wts_in = {}
    for nm in ("Wr1xT", "Wr1bT"):
        wts_in[nm] = nc.dram_tensor(nm, [128, 2 * GDIM], fp16, kind="ExternalInput")
    for nm in ("Wl1T", "Wr2T", "Wl2T"):
        wts_in[nm] = nc.dram_tensor(nm, [128, 4 * GDIM], fp16, kind="ExternalInput")
    b1_in = nc.dram_tensor("b1T", [1, GDIM], fp16, kind="ExternalInput")
    b2_in = nc.dram_tensor("b2T", [1, GDIM], fp16, kind="ExternalInput")

    lap_nch, conv_nch = lap_struct["nch"], conv_struct["nch"]
    lap_idx_in, conv_idx_in, lap_dstl_in, lap_w_in, conv_dstl_in = [], [], [], [], []
    for h in (0, 1):
        lap_idx_in.append(
            nc.dram_tensor(f"lap_idx{h}", [128, lap_nch[h] * 8], i16, kind="ExternalInput")
        )
        conv_idx_in.append(
            nc.dram_tensor(f"conv_idx{h}", [128, conv_nch[h] * 8], i16, kind="ExternalInput")
        )
        lap_dstl_in.append(
            nc.dram_tensor(f"lap_dstl{h}", [128, lap_nch[h]], fp16, kind="ExternalInput")
        )
        lap_w_in.append(
            nc.dram_tensor(f"lap_w{h}", [128, lap_nch[h]], fp16, kind="ExternalInput")
        )
        conv_dstl_in.append(
            nc.dram_tensor(f"conv_dstl{h}", [128, conv_nch[h]], fp16, kind="ExternalInput")
        )

    o_pool = nc.dram_tensor("o_pool", [128, GDIM], f32, kind="ExternalOutput")
    kdump = os.environ.get("KDUMP") == "1"
    dumps = {}
    if kdump:
        dumps["o_xc"] = nc.dram_tensor("o_xc", [NLOC, 2 * D], fp16, kind="ExternalOutput")
        dumps["o_m1"] = nc.dram_tensor("o_m1", [NLOC, GDIM], fp16, kind="ExternalOutput")
        dumps["o_h"] = nc.dram_tensor("o_h", [NLOC, GDIM], fp16, kind="ExternalOutput")
        dumps["o_m2"] = nc.dram_tensor("o_m2", [NLOC, GDIM], fp16, kind="ExternalOutput")
        dumps["o_r"] = nc.dram_tensor("o_r", [128, NWIN * GDIM], fp16, kind="ExternalOutput")

    # ---- internal DRAM ----------------------------------------------------
    xcomb_own = nc.dram_tensor("xcomb_own", [NLOC, 2 * D], fp16)
    xcomb_full = nc.dram_tensor("xcomb_full", [N, 2 * D], fp16, addr_space="Shared")
    h_own = nc.dram_tensor("h_own", [NLOC, GDIM], fp16)
    h_full = nc.dram_tensor("h_full", [N, GDIM], fp16, addr_space="Shared")
    m1_dram = nc.dram_tensor("m1_dram", [NLOC, GDIM], fp16)
    m2_dram = nc.dram_tensor("m2_dram", [NLOC, GDIM], fp16)

    RG = [list(range(NCORES))]

    with TileContext(nc) as tc:
        with (
            tc.tile_pool(name="const", bufs=1) as cpool,
            tc.tile_pool(name="mlap", bufs=3) as mlap,
            tc.tile_pool(name="mconv", bufs=3) as mconv,
            tc.tile_pool(name="asg", bufs=4) as apool,
            tc.tile_pool(name="tT", bufs=8) as tpool,
            tc.tile_pool(name="lT", bufs=2) as lTpool,
            tc.tile_pool(name="o16", bufs=4) as opool,
            tc.tile_pool(name="gl", bufs=2) as gpool,
            tc.tile_pool(name="of32", bufs=1) as f32pool,
            tc.tile_pool(name="pagg", bufs=2, space="PSUM") as pagg,
            tc.tile_pool(name="pbig", bufs=2, space="PSUM") as pbig,
            tc.tile_pool(name="ptr", bufs=2, space="PSUM") as ptrp,
            tc.tile_pool(name="ppool", bufs=1, space="PSUM") as ppool,
        ):
            # ---- critical-path constants on sync queue -------------------
            iota = cpool.tile([128, 128], fp16, tag="iota")
            nc.sync.dma_start(out=iota[:], in_=iota_in[:])
            lap_idx_t, conv_idx_t, lap_dstl_t, lap_w_t, conv_dstl_t = [], [], [], [], []
            for h in (0, 1):
                t = cpool.tile([128, lap_nch[h] * 8], i16, tag=f"lidx{h}")
                nc.sync.dma_start(out=t[:], in_=lap_idx_in[h][:])
                lap_idx_t.append(t)
                t = cpool.tile([128, lap_nch[h]], fp16, tag=f"ldstl{h}")
                nc.sync.dma_start(out=t[:], in_=lap_dstl_in[h][:])
                lap_dstl_t.append(t)
                t = cpool.tile([128, lap_nch[h]], fp16, tag=f"lw{h}")
                nc.sync.dma_start(out=t[:], in_=lap_w_in[h][:])
                lap_w_t.append(t)
            for h in (0, 1):
                t = cpool.tile([128, conv_nch[h] * 8], i16, tag=f"cidx{h}")
                nc.sync.dma_start(out=t[:], in_=conv_idx_in[h][:])
                conv_idx_t.append(t)
                t = cpool.tile([128, conv_nch[h]], fp16, tag=f"cdstl{h}")
                nc.sync.dma_start(out=t[:], in_=conv_dstl_in[h][:])
                conv_dstl_t.append(t)

            # ---- bulk constants on scalar queue --------------------------
            xw_all = cpool.tile([128, NWIN, D], fp16, tag="xw")
            nc.scalar.dma_start(out=xw_all[:], in_=xw_in[:])
            wt = {}
            for nm, wd in (("Wr1xT", 2), ("Wr1bT", 2), ("Wl1T", 4),
                           ("Wr2T", 4), ("Wl2T", 4)):
                t = cpool.tile([128, wd * GDIM], fp16, tag=nm)
                nc.scalar.dma_start(out=t[:], in_=wts_in[nm][:])
                wt[nm] = t
            ones1 = cpool.tile([1, 128], fp16, tag="ones1")
            nc.scalar.dma_start(out=ones1[:], in_=ones_in[:])
            ident = cpool.tile([128, 128], fp16, tag="ident")
            nc.scalar.dma_start(out=ident[:], in_=ident_in[:])
            b1t = cpool.tile([1, GDIM], fp16, tag="b1")
            nc.scalar.dma_start(out=b1t[:], in_=b1_in[:])
            b2t = cpool.tile([1, GDIM], fp16, tag="b2")
            nc.scalar.dma_start(out=b2t[:], in_=b2_in[:])
            # x-half of xcomb (DRAM->DRAM, overlapped with lap gathers)
            nc.scalar.dma_start(out=xcomb_own.ap()[:, 0:D], in_=x16own.ap())
            cinv_t = cpool.tile([128, NWIN], f32, tag="cinv")
            nc.scalar.dma_start(out=cinv_t[:], in_=cinv_in[:])
            pdstl = cpool.tile([128, NWIN], fp16, tag="pdstl")
            nc.scalar.dma_start(out=pdstl[:], in_=pdstl_in[:])

            # pool one-hot: [128, NWIN, 128]
            pool_asg = cpool.tile([128, NWIN, 128], fp16, tag="pasg")
            nc.vector.tensor_tensor(
                out=pool_asg[:],
                in0=pdstl[:].to_broadcast([128, NWIN, 128]),
                in1=iota[:, None, :].to_broadcast([128, NWIN, 128]),
                op=mybir.AluOpType.is_equal,
            )

            # persistent root-term buffer (r1 then overwritten in place by r2)
            r_sbuf = cpool.tile([128, NWIN, GDIM], fp16, tag="rterm")

            qctr = [0]

            class AggPlan:
                """Just-in-time gather + assign-build for one chunk-stream pass."""

                def __init__(self, struct, idx_tiles, tabs, elem, grp, pool,
                             dstl_tiles, w_tiles):
                    self.struct = struct
                    self.idx_tiles = idx_tiles
                    self.tabs = tabs            # (table0_ap, table1_ap)
                    self.elem = elem
                    self.grp = grp
                    self.pool = pool
                    self.dstl_tiles = dstl_tiles
                    self.w_tiles = w_tiles
                    self.msgs = {}
                    self.asg = {}

                def _ensure(self, h, g):
                    if (h, g) in self.msgs:
                        return
                    c0 = g * self.grp
                    cn = min(self.grp, self.struct["nch"][h] - c0)
                    ni = cn * 128
                    tile = self.pool.tile([128, self.grp, self.elem], fp16, tag="msgs")
                    nc.gpsimd.dma_gather(
                        out_ap=tile[:, 0:cn, :],
                        in_ap=self.tabs[h],
                        idxs_ap=self.idx_tiles[h][:, c0 * 8 : (c0 + cn) * 8],
                        num_idxs=ni,
                        num_idxs_reg=ni,
                        elem_size=self.elem,
                        single_packet=single_packet,
                        queue_num=qctr[0] % 4,
                    )
                    qctr[0] += 1
                    self.msgs[(h, g)] = tile
                    t = apool.tile([128, self.grp, 128], fp16, tag="asg")
                    nc.vector.tensor_tensor(
                        out=t[:, 0:cn, :],
                        in0=self.dstl_tiles[h][:, c0 : c0 + cn].to_broadcast(
                            [128, cn, 128]
                        ),
                        in1=iota[:, None, :].to_broadcast([128, cn, 128]),
                        op=mybir.AluOpType.is_equal,
                    )
                    if self.w_tiles is not None:
                        nc.vector.tensor_tensor(
                            out=t[:, 0:cn, :],
                            in0=t[:, 0:cn, :],
                            in1=self.w_tiles[h][:, c0 : c0 + cn].to_broadcast(
                                [128, cn, 128]
                            ),
                            op=mybir.AluOpType.mult,
                        )
                    self.asg[(h, g)] = t

                def chunk(self, ci, h):
                    g, s = ci // self.grp, ci % self.grp
                    self._ensure(h, g)
                    return self.asg[(h, g)][:, s, :], self.msgs[(h, g)][:, s, :]

            def agg_window(plan, w, fdim, tag):
                """Accumulate one window's chunks into a PSUM tile region."""
                cpw, base = plan.struct["cpw"], plan.struct["base"]
                ps = pagg.tile([128, GDIM], f32, tag=tag)
                total = int(cpw[w, 0] + cpw[w, 1])
                k = 0
                for h in (0, 1):
                    for j in range(int(cpw[w, h])):
                        ci = int(base[w, h]) + j
                        asg_ap, msg_ap = plan.chunk(ci, h)
                        nc.tensor.matmul(
                            out=ps[:, 0:fdim],
                            lhsT=asg_ap,
                            rhs=msg_ap,
                            start=(k == 0),
                            stop=(k == total - 1),
                        )
                        k += 1
                return ps

            def wslice(nm, k):
                return wt[nm][:, k * GDIM : (k + 1) * GDIM]

            # ================= Phase L: lap agg + r1 x-part ===============
            with nc.named_scope("lap"):
                lap_plan = AggPlan(
                    lap_struct, lap_idx_t,
                    (x16.ap()[0:T0_END, :], x16.ap()[T1_OFF:N, :]),
                    D, GRP, mlap, lap_dstl_t, lap_w_t,
                )

                xT_loads = {}

                def issue_xT_loads(b):
                    xT = []
                    for k in (0, 1):
                        t = tpool.tile([128, 512], fp16, tag="tT")
                        nc.scalar.dma_start(
                            out=t[:],
                            in_=xT16[k * 128 : (k + 1) * 128, b * 512 : (b + 1) * 512],
                        )
                        xT.append(t)
                    xT_loads[b] = xT

                def r1x_block(b):
                    # r1 = x@(Wr1a+Wr1b)^T + lapraw@Wr1b^T + b1; this is the
                    # x part (runs in lap-phase PE idle, before AG1)
                    xT = xT_loads.pop(b)
                    for nt in range(4):
                        nsl = slice(nt * 128, (nt + 1) * 128)
                        ps = pbig.tile([128, GDIM], f32, tag="pbig")
                        for k in (0, 1):
                            nc.tensor.matmul(
                                out=ps[:], lhsT=xT[k][:, nsl],
                                rhs=wt["Wr1xT"][:, k * GDIM : (k + 1) * GDIM],
                                start=(k == 0), stop=False,
                            )
                        nc.tensor.matmul(
                            out=ps[:], lhsT=ones1[:], rhs=b1t[:],
                            start=False, stop=True,
                        )
                        nc.vector.tensor_copy(r_sbuf[:, b * 4 + nt, :], ps[:])

                issue_xT_loads(0)
                for w in range(NWIN):
                    ps = agg_window(lap_plan, w, D, "pagg")
                    # store lap_x = L_off@x + x; keep RAW L_off@x in SBUF for
                    # the TensorE transposes of the r1 lap part
                    lt = opool.tile([128, D], fp16, tag="lt")
                    nc.vector.tensor_tensor(
                        out=lt[:],
                        in0=ps[:, 0:D],
                        in1=xw_all[:, w, :],
                        op=mybir.AluOpType.add,
                    )
                    nc.sync.dma_start(
                        out=xcomb_own.ap()[w * 128 : (w + 1) * 128, D : 2 * D],
                        in_=lt[:],
                    )
                    nc.vector.tensor_copy(xw_all[:, w, :], ps[:, 0:D])
                    if w % 4 == 0 and w >= 4:
                        issue_xT_loads(w // 4 - 1)
                    if w % 4 == 3 and w >= 7:
                        r1x_block(w // 4 - 1)
                issue_xT_loads(NBLK - 1)
                r1x_block(NBLK - 1)

            # ================= AG1 || r1 lap part =========================
            if phases >= 2:
                nc.gpsimd.collective_compute(
                    "AllGather",
                    mybir.AluOpType.bypass,
                    replica_groups=RG,
                    ins=[xcomb_own.ap().opt()],
                    outs=[xcomb_full.ap().opt()],
                )

            with nc.named_scope("r1"):
                # lap part of r1: pure SBUF compute (TensorE transposes of the
                # raw lap agg + matmuls) -> safely overlaps the collective
                def r1lap_block(b):
                    lT = lTpool.tile([128, 2, 512], fp16, tag="lT")
                    for j in range(4):
                        for k in (0, 1):
                            pt = ptrp.tile([128, 128], fp16, tag="ptr")
                            nc.tensor.transpose(
                                pt[:],
                                xw_all[:, b * 4 + j, k * 128 : (k + 1) * 128],
                                ident[:],
                            )
                            nc.vector.tensor_copy(
                                lT[:, k, j * 128 : (j + 1) * 128], pt[:]
                            )
                    for nt in range(4):
                        nsl = slice(nt * 128, (nt + 1) * 128)
                        ps = pbig.tile([128, GDIM], f32, tag="pbig")
                        for k in (0, 1):
                            nc.tensor.matmul(
                                out=ps[:], lhsT=lT[:, k, nsl],
                                rhs=wt["Wr1bT"][:, k * GDIM : (k + 1) * GDIM],
                                start=(k == 0), stop=(k == 1),
                            )
                        nc.vector.tensor_tensor(
                            out=r_sbuf[:, b * 4 + nt, :],
                            in0=ps[:],
                            in1=r_sbuf[:, b * 4 + nt, :],
                            op=mybir.AluOpType.add,
                        )

                for b in range(NBLK):
                    r1lap_block(b)

            # ================= Phase C1: conv1 agg + l1 dense =============
            if phases >= 3:
                with nc.named_scope("conv1"):
                    c1_plan = AggPlan(
                        conv_struct, conv_idx_t,
                        (xcomb_full.ap()[0:T0_END, :], xcomb_full.ap()[T1_OFF:N, :]),
                        2 * D, GRP, mconv, conv_dstl_t, None,
                    )

                    mT_loads = {}

                    def issue_mT_loads(b, dram, store):
                        tiles = []
                        for k in range(4):
                            t = tpool.tile([128, 512], fp16, tag="tT")
                            nc.sync.dma_start_transpose(
                                out=t[:],
                                in_=dram[b * 512 : (b + 1) * 512,
                                         k * 128 : (k + 1) * 128],
                            )
                            tiles.append(t)
                        store[b] = tiles

                    def c1_window(w):
                        ps = agg_window(c1_plan, w, GDIM, "pagg")
                        mt = opool.tile([128, GDIM], fp16, tag="o16")
                        nc.vector.tensor_tensor(
                            out=mt[:],
                            in0=ps[:],
                            in1=cinv_t[:, w : w + 1].to_broadcast([128, GDIM]),
                            op=mybir.AluOpType.mult,
                        )
                        nc.sync.dma_start(
                            out=m1_dram[w * 128 : (w + 1) * 128, :], in_=mt[:]
                        )

                    def l1_dense(b):
                        mT = mT_loads.pop(b)
                        for nt in range(4):
                            nsl = slice(nt * 128, (nt + 1) * 128)
                            ps = pbig.tile([128, GDIM], f32, tag="pbig")
                            for k in range(4):
                                nc.tensor.matmul(
                                    out=ps[:], lhsT=mT[k][:, nsl],
                                    rhs=wslice("Wl1T", k),
                                    start=(k == 0), stop=(k == 3),
                                )
                            gt = gpool.tile([128, GDIM], fp16, tag="gl")
                            nc.vector.tensor_tensor(
                                out=gt[:], in0=ps[:],
                                in1=r_sbuf[:, b * 4 + nt, :],
                                op=mybir.AluOpType.add,
                            )
                            ht = opool.tile([128, GDIM], fp16, tag="o16")
                            nc.scalar.activation(
                                ht[:], gt[:], mybir.ActivationFunctionType.Gelu
                            )
                            nc.scalar.dma_start(
                                out=h_own[b * 512 + nt * 128 : b * 512 + (nt + 1) * 128, :],
                                in_=ht[:],
                            )

                    for step in range(NBLK + 2):
                        if step < NBLK:
                            for j in range(4):
                                c1_window(step * 4 + j)
                        if step >= 2:
                            l1_dense(step - 2)
                        if 1 <= step <= NBLK:
                            issue_mT_loads(step - 1, m1_dram, mT_loads)

            # ================= AG2 || r2 dense ============================
            if phases >= 4:
                nc.gpsimd.collective_compute(
                    "AllGather",
                    mybir.AluOpType.bypass,
                    replica_groups=RG,
                    ins=[h_own.ap().opt()],
                    outs=[h_full.ap().opt()],
                )

            if phases >= 3:
                with nc.named_scope("r2"):
                    hN_loads = {}

                    def issue_hN_loads(b):
                        tiles = []
                        for j in range(4):
                            t = tpool.tile([128, 512], fp16, tag="tT")
                            nc.scalar.dma_start(
                                out=t[:],
                                in_=h_own[b * 512 + j * 128 : b * 512 + (j + 1) * 128, :],
                            )
                            tiles.append(t)
                        hN_loads[b] = tiles

                    def r2_block(b):
                        hN = hN_loads.pop(b)
                        hT = lTpool.tile([128, 4, 512], fp16, tag="hT")
                        for j in range(4):
                            for k in range(4):
                                pt = ptrp.tile([128, 128], fp16, tag="ptr")
                                nc.tensor.transpose(
                                    pt[:],
                                    hN[j][:, k * 128 : (k + 1) * 128],
                                    ident[:],
                                )
                                nc.vector.tensor_copy(
                                    hT[:, k, j * 128 : (j + 1) * 128], pt[:]
                                )
                        for nt in range(4):
                            nsl = slice(nt * 128, (nt + 1) * 128)
                            ps = pbig.tile([128, GDIM], f32, tag="pbig")
                            for k in range(4):
                                nc.tensor.matmul(
                                    out=ps[:], lhsT=hT[:, k, nsl],
                                    rhs=wslice("Wr2T", k),
                                    start=(k == 0), stop=False,
                                )
                            nc.tensor.matmul(
                                out=ps[:], lhsT=ones1[:], rhs=b2t[:],
                                start=False, stop=True,
                            )
                            nc.vector.tensor_copy(r_sbuf[:, b * 4 + nt, :], ps[:])

                    issue_hN_loads(0)
                    issue_hN_loads(1)
                    for b in range(NBLK):
                        if b + 2 < NBLK:
                            issue_hN_loads(b + 2)
                        r2_block(b)

            # ================= Phase C2: conv2 agg + dense + pool =========
            if phases >= 5:
                with nc.named_scope("conv2"):
                    c2_plan = AggPlan(
                        conv_struct, conv_idx_t,
                        (h_full.ap()[0:T0_END, :], h_full.ap()[T1_OFF:N, :]),
                        GDIM, GRP, mconv, conv_dstl_t, None,
                    )

                    m2T_loads = {}

                    def issue_m2T_loads(b):
                        tiles = []
                        for k in range(4):
                            t = tpool.tile([128, 512], fp16, tag="tT")
                            nc.sync.dma_start_transpose(
                                out=t[:],
                                in_=m2_dram[b * 512 : (b + 1) * 512,
                                            k * 128 : (k + 1) * 128],
                            )
                            tiles.append(t)
                        m2T_loads[b] = tiles

                    def c2_window(w):
                        ps = agg_window(c2_plan, w, GDIM, "pagg")
                        mt = opool.tile([128, GDIM], fp16, tag="o16")
                        nc.vector.tensor_tensor(
                            out=mt[:],
                            in0=ps[:],
                            in1=cinv_t[:, w : w + 1].to_broadcast([128, GDIM]),
                            op=mybir.AluOpType.mult,
                        )
                        nc.sync.dma_start(
                            out=m2_dram[w * 128 : (w + 1) * 128, :], in_=mt[:]
                        )

                    ps_pool = ppool.tile([128, GDIM], f32, tag="ppool")

                    def l2_dense(b):
                        mT = m2T_loads.pop(b)
                        for nt in range(4):
                            nsl = slice(nt * 128, (nt + 1) * 128)
                            ps = pbig.tile([128, GDIM], f32, tag="pbig")
                            for k in range(4):
                                nc.tensor.matmul(
                                    out=ps[:], lhsT=mT[k][:, nsl],
                                    rhs=wslice("Wl2T", k),
                                    start=(k == 0), stop=(k == 3),
                                )
                            ot = opool.tile([128, GDIM], fp16, tag="o16")
                            nc.vector.tensor_tensor(
                                out=ot[:], in0=ps[:],
                                in1=r_sbuf[:, b * 4 + nt, :],
                                op=mybir.AluOpType.add,
                            )
                            ntg = b * 4 + nt
                            nc.tensor.matmul(
                                out=ps_pool[:],
                                lhsT=pool_asg[:, ntg, :],
                                rhs=ot[:],
                                start=(ntg == 0),
                                stop=(ntg == NWIN - 1),
                            )

                    for step in range(NBLK + 2):
                        if step < NBLK:
                            for j in range(4):
                                c2_window(step * 4 + j)
                        if step >= 2:
                            l2_dense(step - 2)
                        if 1 <= step <= NBLK:
                            issue_m2T_loads(step - 1)

                    out_f = f32pool.tile([128, GDIM], f32, tag="of32")
                    nc.vector.tensor_scalar_mul(out_f[:], ps_pool[:], 1.0 / S)
                    nc.sync.dma_start(out=o_pool[:], in_=out_f[:])

            if phases < 5:
                dbg = f32pool.tile([128, GDIM], f32, tag="of32")
                nc.gpsimd.memset(dbg[:], 0.0)
                nc.sync.dma_start(out=o_pool[:], in_=dbg[:])
            if kdump:
                for nm, dram in (("o_xc", xcomb_own), ("o_m1", m1_dram),
                                 ("o_h", h_own), ("o_m2", m2_dram)):
                    nc.sync.dma_start(out=dumps[nm][:], in_=dram[:])
                nc.sync.dma_start(out=dumps["o_r"][:], in_=r_sbuf[:])

    nc.finalize()
    return nc


LAST_EXEC_NS = None
LAST_SCOPES = None


def _maybe_install_trace_hook():
    """Optional NTFF profiling (KTRACE=1): register the axon profile hook."""
    import sys
    import types

    try:
        from trn_agent_boot.trn_boot import _ntff_profile_via_ctypes

        hook = _ntff_profile_via_ctypes("/opt/axon/libaxon_pjrt.so")
        mod = types.ModuleType("antenv.axon_hooks")
        mod.get_axon_ntff_profile_hook = lambda: hook
        mod.set_axon_ntff_profile_hook = lambda h: None
        sys.modules["antenv.axon_hooks"] = mod
        return True
    except Exception:
        return False


def _pack_w(WT):
    """[in, out] contraction-major -> [128, in/128*out] SBUF-wrapped fp16."""
    WT = np.ascontiguousarray(WT).astype(np.float16)
    kc = WT.shape[0] // 128
    return np.ascontiguousarray(
        WT.reshape(kc, 128, GDIM).transpose(1, 0, 2).reshape(128, kc * GDIM)
    )


def kernel(**inputs):
    import os

    from concourse.bass_utils import run_bass_kernel_spmd

    x = np.asarray(inputs["sub2gene_out"], np.float32).reshape(N, D)
    edge_index = np.asarray(inputs["edge_index"])
    W_l1 = np.asarray(inputs["W_l1"], np.float32)
    W_r1 = np.asarray(inputs["W_r1"], np.float32)
    b1 = np.asarray(inputs["b1"], np.float32)
    W_l2 = np.asarray(inputs["W_l2"], np.float32)
    W_r2 = np.asarray(inputs["W_r2"], np.float32)
    b2 = np.asarray(inputs["b2"], np.float32)

    prep = _host_prep(edge_index)
    lap_struct, lap_pc = prep["lap"]
    conv_struct, conv_pc = prep["conv"]

    nc = _build_program(lap_struct, conv_struct)

    x16 = x.astype(np.float16)
    wts = {
        # r1 = x@(Wr1a+Wr1b)^T + (L_off@x)@Wr1b^T + b1
        "Wr1xT": _pack_w((W_r1[:, 0:D] + W_r1[:, D : 2 * D]).T),
        "Wr1bT": _pack_w(W_r1[:, D : 2 * D].T),
        "Wl1T": _pack_w(W_l1.T),
        "Wr2T": _pack_w(W_r2.T),
        "Wl2T": _pack_w(W_l2.T),
    }
    in_maps = []
    for c in range(NCORES):
        xo = x16[c * NLOC : (c + 1) * NLOC]
        m = {
            "x16": x16,
            "x16own": xo,
            "xT16": np.ascontiguousarray(xo.T),
            "xwrap": np.ascontiguousarray(
                xo.reshape(NWIN, 128, D).transpose(1, 0, 2).reshape(128, NWIN * D)
            ),
            "iota": prep["iota"],
            "ident": np.eye(128, dtype=np.float16),
            "ones1": np.ones((1, 128), np.float16),
            "cinv": prep["cinv"][c],
            "pool_dstl": prep["pool_dstl"],
            "b1T": b1.astype(np.float16)[None, :],
            "b2T": b2.astype(np.float16)[None, :],
            **wts,
        }
        for h in (0, 1):
            m[f"lap_idx{h}"] = lap_pc[c]["idx"][h]
            m[f"lap_dstl{h}"] = lap_pc[c]["dstl"][h]
            m[f"lap_w{h}"] = lap_pc[c]["wgt"][h]
            m[f"conv_idx{h}"] = conv_pc[c]["idx"][h]
            m[f"conv_dstl{h}"] = conv_pc[c]["dstl"][h]
        in_maps.append(m)

    trace = os.environ.get("KTRACE") == "1" and _maybe_install_trace_hook()
    res = run_bass_kernel_spmd(nc, in_maps, core_ids=list(range(NCORES)), trace=trace)
    global LAST_EXEC_NS, LAST_SCOPES, LAST_RESULTS, LAST_RES
    LAST_EXEC_NS = res.exec_time_ns
    LAST_SCOPES = res.per_core_scope_times
    LAST_RESULTS = res.results
    LAST_RES = res
    out = np.concatenate([res.results[c]["o_pool"] for c in range(NCORES)], axis=0)
    return out.astype(np.float32)
